# revision 1
# baseline (speedup 1.0000x reference)
"""Trainium2 Bass kernel for nn_AblatedModel_40802189312754 (2-layer GNN + scoring).

Sharding: entities row-sharded 8 ways (6250/core, padded to 6400); batch
replicated; final [B, N] logits column-sharded by entity shard.

Per core: SpMM = (edge-feature chunks) x (host-built indicator matrices)
accumulated as PE matmuls into PSUM windows of 512 segments, kept transposed
(dim on partitions). Layer-1 edge features are host-pre-gathered and
streamed (DMAs round-robined over 4 engine queues); layer-2 edge features
are dma_gather'ed from an AllGathered bf16 h-table that is rebuilt in
"AG-group-major" order and AllGathered in 3 pieces pipelined behind layer-1
windows, so half-0 gathers start before layer 1 finishes.  All 8 cores share
one instruction stream: bins of 32 segments, chunk count per (window, bin,
table-half) = max over cores.  Scoring GEMM fp32, sigmoid output bf16.
"""
import sys
sys.path.insert(0, '/opt/trn_rl_repo')

import numpy as np
import ml_dtypes

import concourse.bacc as bacc
import concourse.tile as tile
import concourse.mybir as mybir
from concourse.bass_utils import run_bass_kernel_spmd

BF16 = ml_dtypes.bfloat16

N_ENT = 50000
D = 128
B = 1024
NC = 8
SH = 6250            # real entities per shard
NSH = 6400           # padded shard size
NV = NC * NSH        # virtual table rows (51200)
BN_EPS = 1e-5
SEGW = 32            # bin width in segments
WINDOWS = [(w, min(512, NSH - w)) for w in range(0, NSH, 512)]  # 13 windows
NW = len(WINDOWS)
NBIN = NSH // SEGW   # 200 bins
GRP = 4              # chunks per L2 gather group
GIDX = GRP * 128     # idxs per gather
GIW = GIDX // 16     # idx cols per gather
SGRP = 4             # chunks per L1 stream tile
SIDX = SGRP * 128
HALF = 32768
# AG groups: windows 0-3 / 4-7 / 8-12 (per-shard rows 2048/2048/2304)
AGW = [(0, 4), (4, 3), (8, 5)]
AGROWS = [2048, 2048, 2304]
NB_ROWS = NC * AGROWS[2]          # 18432


def _remap(vid):
    """virtual id -> (half, pos within half-table) after AG-group reorder."""
    k = vid // NSH
    local = vid - k * NSH
    w = local // 512
    pos = np.where(
        w <= 3, k * 2048 + local,
        np.where(w <= 7, 16384 + k * 2048 + (local - 2048),
                 HALF + k * 2304 + (local - 4096)))
    return (pos >= HALF).astype(np.int64), np.where(pos >= HALF, pos - HALF, pos)


def _wrap_idx(ids):
    """[n] -> [128, n//16] int16 gather-index layout (wrapped, replicated 8x)."""
    n = len(ids)
    w = ids.reshape(n // 16, 16).T
    return np.ascontiguousarray(np.tile(w, (8, 1)).astype(np.int16))


def _vid(ent):
    owner = ent // SH
    return owner * NSH + (ent - owner * SH)


def _make_plan(rows, cols, vals):
    """Uniform cross-core plan.

    Chunk order: window-major, within a window half 0 chunks then half 1
    (matches L1 streaming).  L2 processes half-major (all h0 windows, then
    all h1 windows) but indexes the same chunk ranges.

    Returns (struct, cores):
      struct['nch'][w][h]   = chunks for (window w, half h)
      struct['chunks'][w][h] = [(seg0, bin_first)] per chunk
      struct['ngrp'][w][h]  = gather groups (GRP chunks each) for L2
      cores[k] = {'idx': [128, ngt*GIW] i16 (L2 groups, half-major),
                  'ind': [128, ncht*SEGW] f32 (chunk-major),
                  'g1src': [ncht, 128] original col id or -1}
    """
    vcol_all = _vid(cols)
    half_all, pos_all = _remap(vcol_all)
    per_core = []
    for k in range(NC):
        m = (rows >= k * SH) & (rows < (k + 1) * SH)
        r = rows[m] - k * SH
        h = half_all[m]
        p = pos_all[m]
        c = cols[m]
        v = vals[m].astype(np.float32)
        key = (r // SEGW) * 2 + h
        o = np.lexsort((p, r, key))
        per_core.append((key[o], r[o], p[o], c[o], v[o]))

    nch_bin = np.zeros((NBIN, 2), np.int64)
    bounds = []
    for k in range(NC):
        key = per_core[k][0]
        lo = np.searchsorted(key, np.arange(NBIN * 2))
        hi = np.searchsorted(key, np.arange(NBIN * 2) + 1)
        bounds.append((lo, hi))
        cnt = (hi - lo).reshape(NBIN, 2)
        nch_bin = np.maximum(nch_bin, -(-cnt // 128))

    struct = {'nch': [], 'chunks': [], 'ngrp': []}
    core_chunks = [[] for _ in range(NC)]   # per core: (a, b) edge ranges
    for wi, (w0, wsz) in enumerate(WINDOWS):
        b0, b1 = w0 // SEGW, (w0 + wsz) // SEGW
        nch_w, chunks_w, ngrp_w = [], [], []
        for h in (0, 1):
            cl = [(b, j) for b in range(b0, b1) for j in range(nch_bin[b, h])]
            nch_w.append(len(cl))
            ngrp_w.append(-(-len(cl) // GRP))
            ch = []
            for (b, j) in cl:
                ch.append(((b - b0) * SEGW, j == 0 and b == b0 and h == 0))
                for k in range(NC):
                    lo, hi = bounds[k]
                    a = lo[b * 2 + h] + j * 128
                    e = min(a + 128, hi[b * 2 + h])
                    core_chunks[k].append((a, max(a, e)))
            chunks_w.append(ch)
        struct['nch'].append(nch_w)
        struct['chunks'].append(chunks_w)
        struct['ngrp'].append(ngrp_w)

    ncht = sum(sum(x) for x in struct['nch'])
    ngt = sum(sum(x) for x in struct['ngrp'])
    struct['ncht'] = ncht
    struct['ngt'] = ngt
    # L1 stream groups: GRP chunks per [128, 512] tile, grouped per window
    struct['ngrp1'] = [-(-sum(struct['nch'][wi]) // SGRP) for wi in range(NW)]
    struct['ngt1'] = sum(struct['ngrp1'])

    # per-core blobs
    cores = []
    for k in range(NC):
        key, r, p, c, v = per_core[k]
        ind = np.zeros((ncht, 128, SEGW), np.float32)
        g1src = np.full((ncht, 128), -1, np.int64)
        pos_chunk = np.zeros((ncht, 128), np.int64)   # remapped gather pos
        ci = 0
        for wi, (w0, wsz) in enumerate(WINDOWS):
            for h in (0, 1):
                for (seg0, _) in struct['chunks'][wi][h]:
                    a, e = core_chunks[k][ci]
                    n = e - a
                    if n:
                        ind[ci, np.arange(n), r[a:e] - w0 - seg0] = v[a:e]
                        g1src[ci, :n] = c[a:e]
                        pos_chunk[ci, :n] = p[a:e]
                    ci += 1
        # L2 gather idx blob: half-major group order
        idx_blocks = []
        for h in (0, 1):
            ci = 0
            for wi in range(NW):
                base0 = sum(sum(struct['nch'][x]) for x in range(wi))
                cbase = base0 + (struct['nch'][wi][0] if h else 0)
                nch = struct['nch'][wi][h]
                ngr = struct['ngrp'][wi][h]
                for g in range(ngr):
                    blk = np.zeros(GIDX, np.int64)
                    for j in range(GRP):
                        cj = g * GRP + j
                        if cj < nch:
                            blk[j * 128:(j + 1) * 128] = pos_chunk[cbase + cj]
                    idx_blocks.append(_wrap_idx(blk))
        cores.append({
            'idx': np.concatenate(idx_blocks, 1),
            'ind': np.ascontiguousarray(
                ind.transpose(1, 0, 2).reshape(128, ncht * SEGW)),
            'g1src': g1src,
        })
    return struct, cores


def _build_nc(struct):
    ncht, ngt = struct['ncht'], struct['ngt']
    maxch_w = max(sum(x) for x in struct['nch'])

    nc = bacc.Bacc("TRN2", target_bir_lowering=False, debug=False,
                   enable_asserts=True, num_devices=NC, num_swdge_queues=4)
    f32, bf, i16 = mybir.dt.float32, mybir.dt.bfloat16, mybir.dt.int16
    AF = mybir.ActivationFunctionType

    g1_d = nc.dram_tensor("g1", [struct['ngt1'], 128, SIDX], bf,
                          kind="ExternalInput")
    ind_d = nc.dram_tensor("ind", [128, ncht * SEGW], bf, kind="ExternalInput")
    idx_d = nc.dram_tensor("idx", [128, ngt * GIW], i16, kind="ExternalInput")
    w1_d = nc.dram_tensor("w1", [D, D], f32, kind="ExternalInput")
    w2_d = nc.dram_tensor("w2", [D, D], f32, kind="ExternalInput")
    w_d = nc.dram_tensor("w", [D, D], f32, kind="ExternalInput")
    bn_d = nc.dram_tensor("bn", [D, 8], f32, kind="ExternalInput")
    et_d = nc.dram_tensor("et", [128, NSH], f32, kind="ExternalInput")
    ebh_d = nc.dram_tensor("ebh", [128, B], f32, kind="ExternalInput")
    rgt_d = nc.dram_tensor("rgt", [128, B], f32, kind="ExternalInput")
    bidx_d = nc.dram_tensor("bidx", [128, 64], i16, kind="ExternalInput")
    ident_d = nc.dram_tensor("ident", [D, D], f32, kind="ExternalInput")
    out_d = nc.dram_tensor("out", [8, NW, 128, 512], bf, kind="ExternalOutput")

    with tile.TileContext(nc) as tc:
        with tc.tile_pool(name="const", bufs=1) as cp, \
             tc.tile_pool(name="gp", bufs=12) as gp, \
             tc.tile_pool(name="indp", bufs=3) as indp, \
             tc.tile_pool(name="idxp", bufs=1) as idxp, \
             tc.tile_pool(name="sp", bufs=1) as spool, \
             tc.tile_pool(name="hp", bufs=1) as hp, \
             tc.tile_pool(name="hep", bufs=8) as hep, \
             tc.tile_pool(name="h2p", bufs=2) as h2p, \
             tc.tile_pool(name="bp", bufs=1) as bp, \
             tc.tile_pool(name="op", bufs=6) as op, \
             tc.tile_pool(name="pch", bufs=2, space="PSUM") as pch, \
             tc.tile_pool(name="px", bufs=2, space="PSUM") as px, \
             tc.tile_pool(name="psc", bufs=2, space="PSUM") as psc, \
             tc.tile_pool(name="dram", bufs=1, space="DRAM") as dp:

            w1_t = cp.tile([D, D], f32); nc.sync.dma_start(w1_t[:], w1_d[:])
            w2_t = cp.tile([D, D], f32); nc.sync.dma_start(w2_t[:], w2_d[:])
            w_t = cp.tile([D, D], f32); nc.sync.dma_start(w_t[:], w_d[:])
            bn_t = cp.tile([D, 8], f32); nc.sync.dma_start(bn_t[:], bn_d[:])
            id_t = cp.tile([D, D], f32); nc.sync.dma_start(id_t[:], ident_d[:])
            ebh_t = cp.tile([128, B], f32); nc.scalar.dma_start(ebh_t[:], ebh_d[:])
            rgt_t = cp.tile([128, B], f32); nc.scalar.dma_start(rgt_t[:], rgt_d[:])
            bidx_t = cp.tile([128, 64], i16); nc.scalar.dma_start(bidx_t[:], bidx_d[:])
            zl_t = cp.tile([1, 128], bf); nc.vector.memset(zl_t[:], 0.0)
            zr_t = cp.tile([1, 512], bf); nc.vector.memset(zr_t[:], 0.0)

            idx_all = idxp.tile([128, ngt * GIW], i16)
            nc.sync.dma_start(idx_all[:], idx_d[:])
            s_t = spool.tile([128, NSH], f32, tag="s")
            h1t_t = hp.tile([128, NSH], bf, tag="h1t")
            fet_t = hp.tile([128, NSH], f32, tag="fet")
            h2tb_t = hp.tile([128, NSH], bf, tag="h2tb")

            hsh1 = dp.tile([NSH, D], bf, tag="hsh1")
            hsh2 = dp.tile([NSH, D], bf, tag="hsh2")
            hfa0_sh = dp.tile([16384, D], bf, tag="hfa0s", addr_space="Shared")
            hfa1_sh = dp.tile([16384, D], bf, tag="hfa1s", addr_space="Shared")
            hfa = dp.tile([HALF, D], bf, tag="hfa")
            hfb = dp.tile([NB_ROWS, D], bf, tag="hfb", addr_space="Shared")

            engs = [nc.sync, nc.scalar]
            rr = [0]

            def rr_eng():
                e = engs[rr[0] % 2]
                rr[0] += 1
                return e

            def cbase_of(wi, h):
                base = sum(sum(struct['nch'][x]) for x in range(wi))
                return base + (struct['nch'][wi][0] if h else 0)

            # ---------- layer 1: stream pre-gathered chunks ----------
            def tail1(wi):
                # xform/store for window wi, deferred one window to keep the
                # PE queue free of round-trip stalls
                w0, wsz = WINDOWS[wi]
                xp = px.tile([128, 512], f32, tag="xp", name=f"xp1_{wi}")
                nc.tensor.matmul(xp[:, :wsz], w1_t[:], s_t[:, w0:w0 + wsz],
                                 start=True, stop=True)
                nc.scalar.activation(h1t_t[:, w0:w0 + wsz], xp[:, :wsz],
                                     AF.Relu, bias=bn_t[:, 0:1], scale=1.0)
                for t in range(wsz // 128):
                    hent = hep.tile([128, 128], bf, tag="hent")
                    nc.sync.dma_start_transpose(
                        hent[:], h1t_t[:, w0 + t * 128:w0 + (t + 1) * 128])
                    nc.sync.dma_start(
                        hsh1[w0 + t * 128:w0 + (t + 1) * 128, :], hent[:])
                if wi == 3:
                    nc.gpsimd.collective_compute(
                        "AllGather", mybir.AluOpType.bypass,
                        replica_groups=[list(range(NC))],
                        ins=[hsh1[0:2048].opt()], outs=[hfa0_sh[:].opt()])
                    nc.sync.dma_start(hfa[0:16384, :], hfa0_sh[:])
                elif wi == 7:
                    nc.gpsimd.collective_compute(
                        "AllGather", mybir.AluOpType.bypass,
                        replica_groups=[list(range(NC))],
                        ins=[hsh1[2048:4096].opt()],
                        outs=[hfa1_sh[:].opt()])
                    nc.sync.dma_start(hfa[16384:HALF, :], hfa1_sh[:])

            g1base = 0
            for wi, (w0, wsz) in enumerate(WINDOWS):
                nch0, nch1 = struct['nch'][wi]
                nch = nch0 + nch1
                cbase = cbase_of(wi, 0)
                gts = []
                for g in range(struct['ngrp1'][wi]):
                    g_t = gp.tile([128, SIDX], bf, tag="g",
                                  name=f"g1_w{wi}_{g}")
                    rr_eng().dma_start(g_t[:], g1_d[g1base + g])
                    gts.append(g_t)
                g1base += struct['ngrp1'][wi]
                ind_t = indp.tile([128, maxch_w * SEGW], bf, tag="ind")
                nc.sync.dma_start(
                    ind_t[:, :nch * SEGW],
                    ind_d[:, cbase * SEGW:(cbase + nch) * SEGW])
                ps = pch.tile([128, 512], f32, tag="ps")
                nc.tensor.matmul(ps[:, :wsz], zl_t[:], zr_t[:, :wsz],
                                 start=True, stop=False, skip_group_check=True)
                allch = struct['chunks'][wi][0] + struct['chunks'][wi][1]
                for cj, (seg0, _) in enumerate(allch):
                    nc.tensor.matmul(
                        ps[:, seg0:seg0 + SEGW],
                        gts[cj // SGRP][:, (cj % SGRP) * 128:
                                        (cj % SGRP + 1) * 128],
                        ind_t[:, cj * SEGW:(cj + 1) * SEGW],
                        start=False, stop=(cj == nch - 1),
                        skip_group_check=True)
                nc.vector.tensor_copy(s_t[:, w0:w0 + wsz], ps[:, :wsz])
                tail1(wi)

            # preload et into fet (overlaps with L2 below)
            nc.scalar.dma_start(fet_t[:], et_d[:])

            # ---------- layer 2: gather h1 from AG table, half-major ----
            qn = [0]
            ag2_done = [False]

            def gather_groups(wi, h, gbase):
                ngr = struct['ngrp'][wi][h]
                src = hfa if h == 0 else hfb
                gts = []
                for g in range(ngr):
                    g_t = gp.tile([128, GIDX], bf, tag="g",
                                  name=f"g2_w{wi}_h{h}_{g}")
                    nc.gpsimd.dma_gather(
                        g_t[:].rearrange("p (c e) -> p c e", e=D),
                        src[:], idx_all[:, (gbase + g) * GIW:
                                        (gbase + g + 1) * GIW],
                        GIDX, GIDX, D, queue_num=qn[0] % 4)
                    qn[0] += 1
                    gts.append(g_t)
                return gts

            def tail2(wi):
                w0, wsz = WINDOWS[wi]
                xp = px.tile([128, 512], f32, tag="xp", name=f"xp2_{wi}")
                nc.tensor.matmul(xp[:, :wsz], w2_t[:], s_t[:, w0:w0 + wsz],
                                 start=True, stop=True)
                h2w = h2p.tile([128, 512], f32, tag="h2w")
                nc.scalar.activation(h2w[:, :wsz], xp[:, :wsz],
                                     AF.Relu, bias=bn_t[:, 1:2], scale=1.0)
                nc.vector.tensor_tensor(
                    fet_t[:, w0:w0 + wsz], fet_t[:, w0:w0 + wsz],
                    h2w[:, :wsz], mybir.AluOpType.add)
                nc.vector.tensor_copy(h2tb_t[:, w0:w0 + wsz], h2w[:, :wsz])
                if wi == NW - 1:
                    nc.vector.memset(h2tb_t[:, NSH - 1:NSH], 0.0)
                for t in range(wsz // 128):
                    hent = hep.tile([128, 128], bf, tag="hent")
                    nc.sync.dma_start_transpose(
                        hent[:], h2tb_t[:, w0 + t * 128:w0 + (t + 1) * 128])
                    nc.sync.dma_start(
                        hsh2[w0 + t * 128:w0 + (t + 1) * 128, :], hent[:])

            gbase = 0
            for h in (0, 1):
                for wi, (w0, wsz) in enumerate(WINDOWS):
                    if h == 0 and wi == 4 and not ag2_done[0]:
                        # issue AG2 early: its h1 data is ready at L1 end and
                        # pass-B gathers need it; don't let it queue behind
                        # the whole pass-A gather stream.
                        nc.gpsimd.collective_compute(
                            "AllGather", mybir.AluOpType.bypass,
                            replica_groups=[list(range(NC))],
                            ins=[hsh1[4096:NSH].opt()],
                            outs=[hfb[:].opt()])
                        ag2_done[0] = True
                    nch = struct['nch'][wi][h]
                    cbase = cbase_of(wi, h)
                    gts = gather_groups(wi, h, gbase)
                    gbase += struct['ngrp'][wi][h]
                    ind_t = indp.tile([128, maxch_w * SEGW], bf, tag="ind")
                    nc.sync.dma_start(
                        ind_t[:, :nch * SEGW],
                        ind_d[:, cbase * SEGW:(cbase + nch) * SEGW])
                    ps = pch.tile([128, 512], f32, tag="ps")
                    nc.tensor.matmul(ps[:, :wsz], zl_t[:], zr_t[:, :wsz],
                                     start=True, stop=False,
                                     skip_group_check=True)
                    for cj, (seg0, _) in enumerate(struct['chunks'][wi][h]):
                        nc.tensor.matmul(
                            ps[:, seg0:seg0 + SEGW],
                            gts[cj // GRP][:, (cj % GRP) * 128:
                                           (cj % GRP + 1) * 128],
                            ind_t[:, cj * SEGW:(cj + 1) * SEGW],
                            start=False, stop=(cj == nch - 1),
                            skip_group_check=True)
                    if h == 0:
                        nc.vector.tensor_copy(s_t[:, w0:w0 + wsz], ps[:, :wsz])
                    else:
                        nc.vector.tensor_tensor(
                            s_t[:, w0:w0 + wsz], s_t[:, w0:w0 + wsz],
                            ps[:, :wsz], mybir.AluOpType.add)
                        tail2(wi)

            # ---------- batch tail ----------
            tlo = bp.tile([128, B], bf)
            nc.gpsimd.dma_gather(
                tlo[:].rearrange("p (c e) -> p c e", e=D), hsh2[:],
                bidx_t[:, 0:64], 1024, 1024, D, queue_num=0)
            xpart = bp.tile([128, B], f32)
            nc.vector.tensor_copy(xpart[:], tlo[:])
            xin_dram = dp.tile([128, B], f32, tag="xin")
            xout_dram = dp.tile([128, B], f32, tag="xout", addr_space="Shared")
            nc.sync.dma_start(xin_dram[:], xpart[:])
            nc.gpsimd.collective_compute(
                "AllReduce", mybir.AluOpType.add,
                replica_groups=[list(range(NC))],
                ins=[xin_dram[:].opt()], outs=[xout_dram[:].opt()])
            xraw = bp.tile([128, B], f32)
            nc.sync.dma_start(xraw[:], xout_dram[:])
            nc.vector.tensor_tensor(xraw[:], xraw[:], ebh_t[:],
                                    mybir.AluOpType.add)
            xtb = bp.tile([128, B], f32)
            for j in range(8):
                tp = px.tile([128, 128], f32, tag="xp")
                nc.tensor.transpose(tp[:], xraw[:, j * 128:(j + 1) * 128],
                                    id_t[:])
                nc.vector.tensor_scalar(
                    xtb[:, j * 128:(j + 1) * 128], tp[:],
                    bn_t[:, 2:3], bn_t[:, 3:4],
                    mybir.AluOpType.mult, mybir.AluOpType.add)
            vmt = bp.tile([128, B], f32)
            for hb in range(2):
                sl = slice(hb * 512, hb * 512 + 512)
                wmp = px.tile([128, 512], f32, tag="xp")
                nc.tensor.matmul(wmp[:], w_t[:], rgt_t[:, sl],
                                 start=True, stop=True)
                nc.vector.tensor_tensor(vmt[:, sl], xtb[:, sl], wmp[:],
                                        mybir.AluOpType.mult)
            nc.vector.tensor_scalar(vmt[:], vmt[:], bn_t[:, 4:5], bn_t[:, 5:6],
                                    mybir.AluOpType.mult, mybir.AluOpType.add)

            # ---------- scoring ----------
            for bt in range(8):
                for wi, (w0, wsz) in enumerate(WINDOWS):
                    sc = psc.tile([128, 512], f32, tag="sc")
                    nc.tensor.matmul(sc[:, :wsz],
                                     vmt[:, bt * 128:(bt + 1) * 128],
                                     fet_t[:, w0:w0 + wsz],
                                     start=True, stop=True)
                    ob = op.tile([128, 512], bf, tag="ob")
                    nc.scalar.activation(ob[:, :wsz], sc[:, :wsz], AF.Sigmoid)
                    rr_eng().dma_start(out_d[bt, wi, :, :wsz], ob[:, :wsz])
    nc.compile()
    return nc


def _host_prep(inputs):
    rows = np.asarray(inputs["adj_rows"]).astype(np.int64)
    cols = np.asarray(inputs["adj_cols"]).astype(np.int64)
    vals = np.asarray(inputs["adj_vals"], np.float32)
    E = np.asarray(inputs["E_emb"], np.float32)[np.asarray(inputs["init_ind"])]
    E_bf = E.astype(BF16)
    bh = np.asarray(inputs["batch_head"]).astype(np.int64)
    rel = np.asarray(inputs["batch_rel"]).astype(np.int64)
    R = np.asarray(inputs["R_emb"], np.float32)

    g0 = np.asarray(inputs["bn0_gamma"], np.float32) / np.sqrt(1.0 + BN_EPS)
    b0 = np.asarray(inputs["bn0_beta"], np.float32)
    g1 = np.asarray(inputs["bn1_gamma"], np.float32) / np.sqrt(1.0 + BN_EPS)
    b1v = np.asarray(inputs["bn1_beta"], np.float32)
    bn = np.ascontiguousarray(np.stack(
        [np.asarray(inputs["b1"], np.float32),
         np.asarray(inputs["b2"], np.float32),
         g0, b0, g1, b1v,
         np.zeros(D, np.float32), np.zeros(D, np.float32)], axis=1))

    bh_owner = bh // SH
    bh_local = bh - bh_owner * SH

    def slot_layout(a):          # [1024, D] -> [128, 8*D], slot i=(p,j)->j*128+p
        return np.ascontiguousarray(
            a.reshape(8, 128, D).transpose(1, 0, 2).reshape(128, 8 * D))

    ebh_l = slot_layout(E[bh])
    rgt = np.ascontiguousarray(R[rel].T.astype(np.float32))

    struct, cores = _make_plan(rows, cols, vals)
    ncht = struct['ncht']

    # chunk -> (window-local group slot) map for the L1 stream blob
    ngt1 = struct['ngt1']
    g1_row = np.zeros(ncht, np.int64)
    g1_col = np.zeros(ncht, np.int64)
    ci = 0
    g1base = 0
    for wi in range(NW):
        nch = sum(struct['nch'][wi])
        for cj in range(nch):
            g1_row[ci] = g1base + cj // SGRP
            g1_col[ci] = cj % SGRP
            ci += 1
        g1base += struct['ngrp1'][wi]

    in_maps = []
    for k in range(NC):
        pl = cores[k]
        g1_blob = np.zeros((ngt1, 128, SGRP, 128), BF16)
        srcc = pl['g1src']
        m = srcc >= 0
        chunk_rows = np.broadcast_to(g1_row[:, None], srcc.shape)[m]
        chunk_cols = np.broadcast_to(g1_col[:, None], srcc.shape)[m]
        slot = np.broadcast_to(np.arange(128)[None, :], srcc.shape)[m]
        g1_blob[chunk_rows, slot, chunk_cols] = E_bf[srcc[m]]
        g1_blob = np.ascontiguousarray(g1_blob.reshape(ngt1, 128, SGRP * 128))
        et = np.zeros((D, NSH), np.float32)
        et[:, :SH] = E[k * SH:(k + 1) * SH].T
        in_maps.append({
            "g1": g1_blob,
            "ind": pl['ind'].astype(BF16),
            "idx": pl['idx'],
            "w1": np.asarray(inputs["W1"], np.float32),
            "w2": np.asarray(inputs["W2"], np.float32),
            "w": np.asarray(inputs["W"], np.float32),
            "bn": bn, "et": et, "ebh": ebh_l, "rgt": rgt,
            "bidx": _wrap_idx(np.where(bh_owner == k, bh_local, NSH - 1)),
            "ident": np.eye(D, dtype=np.float32),
        })
    return struct, in_maps


def _run(inputs, trace=False):
    struct, in_maps = _host_prep(inputs)
    nc = _build_nc(struct)
    res = run_bass_kernel_spmd(nc, in_maps, core_ids=list(range(NC)),
                               trace=trace)
    outs = []
    for k in range(NC):
        o = res.results[k]["out"]            # [8, NW, 128, 512] bf16
        o = o.transpose(0, 2, 1, 3).reshape(B, NW * 512)[:, :NSH]
        outs.append(o[:, :SH])
    return np.concatenate(outs, axis=1).astype(np.float32), res


def kernel(**inputs):
    out, _ = _run(inputs, trace=False)
    return out



# revision 9
# speedup vs baseline: 1.1063x; 1.1063x over previous
"""Trainium2 Bass kernel for nn_AblatedModel_40802189312754 (2-layer GNN + scoring).

Sharding: entities row-sharded 8 ways (6250/core, padded to 6400); batch
replicated; final [B, N] logits column-sharded by entity shard.

v2 design (descriptor-count optimized):
- L1 edge features are host-pre-gathered into a PARTITION-MAJOR blob
  [128, ncht*128] so each per-window load is one DMA with one large
  descriptor per partition (~390 GB/s instead of ~110).
- SpMM chunks are 128 edges x 64 segments (SEGW=64) to cut gather rows.
- L2 gathers read the AllGathered bf16 h1 table directly from the Shared
  collective-output tiles (no staging copies); the h0 table is one
  [32768, D] Shared tile filled by two AllGathers.
- All collective triggers are issued on gpsimd BEFORE the gather stream so
  the gather pipeline never blocks on a mid-stream collective wait.
- h1/h2 entity-major DRAM tables are produced via PE transposes (frees the
  Sync engine's HWDGE ring from descriptor-heavy DMA_TRANSPOSEs).
- Scoring output assembled per window [128, 8*512] and stored with one DMA
  per window into a partition-major out blob.
- All 8 cores share one instruction stream: per-(window,half) chunk counts
  are the max over cores; gather padding uses trailing -1 (skipped) where
  safe.
"""
import sys
sys.path.insert(0, '/opt/trn_rl_repo')

import numpy as np
import ml_dtypes

import concourse.bacc as bacc
import concourse.tile as tile
import concourse.mybir as mybir
from concourse.bass_utils import run_bass_kernel_spmd

BF16 = ml_dtypes.bfloat16

N_ENT = 50000
D = 128
B = 1024
NC = 8
SH = 6250            # real entities per shard
NSH = 6400           # padded shard size
BN_EPS = 1e-5
SEGW = 64            # bin width in segments
WINDOWS = [(w, min(512, NSH - w)) for w in range(0, NSH, 512)]  # 13 windows
NW = len(WINDOWS)
NBIN = NSH // SEGW   # 100 bins
GRP = 8              # chunks per L2 gather op
GIDX = GRP * 128     # idxs per gather op
GIW = GIDX // 16     # idx cols per gather op
HALF = 32768
NB_ROWS = NC * 2304  # 18432 (half-1 table rows)
GP_BUFS = 12         # L2 gather tile pool depth


def _remap(vid):
    """virtual id -> (half, pos within half-table) after AG reorder.

    h0 table = one AllGather of each core's hsh1[0:4096] (L1 windows 0-7);
    h1 table = one AllGather of hsh1[4096:6400] (windows 8-12).
    """
    k = vid // NSH
    local = vid - k * NSH
    w = local // 512
    pos = np.where(w <= 7, k * 4096 + local,
                   HALF + k * 2304 + (local - 4096))
    return (pos >= HALF).astype(np.int64), np.where(pos >= HALF, pos - HALF, pos)


def _wrap_idx(ids):
    """[n] -> [128, n//16] int16 gather-index layout (wrapped, replicated 8x)."""
    n = len(ids)
    w = ids.reshape(n // 16, 16).T
    return np.ascontiguousarray(np.tile(w, (8, 1)).astype(np.int16))


def _vid(ent):
    owner = ent // SH
    return owner * NSH + (ent - owner * SH)


def _make_plan(rows, cols, vals):
    """Uniform cross-core plan.

    Chunk order: window-major, within a window half 0 chunks then half 1
    (matches L1 streaming and the ind blob).  L2 processes half-major (all
    h0 windows, then all h1 windows) but indexes the same chunk ranges.

    Returns (struct, cores):
      struct['nch'][w][h]   = chunks for (window w, half h)
      struct['chunks'][w][h] = [seg0] per chunk (window-local segment base)
      struct['ngrp'][w][h]  = gather ops (GRP chunks each) for L2
      cores[k] = {'idx': [128, ngt*GIW] i16 (L2 ops, half-major),
                  'ind': [128, ncht*SEGW] bf16 (chunk-major),
                  'g1':  [128, ncht*128] bf16 (lane-major L1 features)}
    """
    vcol_all = _vid(cols)
    half_all, pos_all = _remap(vcol_all)
    per_core = []
    for k in range(NC):
        m = (rows >= k * SH) & (rows < (k + 1) * SH)
        r = rows[m] - k * SH
        h = half_all[m]
        p = pos_all[m]
        c = cols[m]
        v = vals[m].astype(np.float32)
        key = (r // SEGW) * 2 + h
        o = np.lexsort((p, r, key))
        per_core.append((key[o], r[o], p[o], c[o], v[o]))

    nch_bin = np.zeros((NBIN, 2), np.int64)
    bounds = []
    for k in range(NC):
        key = per_core[k][0]
        lo = np.searchsorted(key, np.arange(NBIN * 2))
        hi = np.searchsorted(key, np.arange(NBIN * 2) + 1)
        bounds.append((lo, hi))
        cnt = (hi - lo).reshape(NBIN, 2)
        nch_bin = np.maximum(nch_bin, -(-cnt // 128))

    struct = {'nch': [], 'chunks': [], 'ngrp': []}
    core_chunks = [[] for _ in range(NC)]   # per core: (a, b) edge ranges
    for wi, (w0, wsz) in enumerate(WINDOWS):
        b0, b1 = w0 // SEGW, (w0 + wsz) // SEGW
        nch_w, chunks_w, ngrp_w = [], [], []
        for h in (0, 1):
            cl = [(b, j) for b in range(b0, b1) for j in range(nch_bin[b, h])]
            nch_w.append(len(cl))
            ngrp_w.append(-(-len(cl) // GRP)) if cl else ngrp_w.append(0)
            ch = []
            for (b, j) in cl:
                ch.append((b - b0) * SEGW)
                for k in range(NC):
                    lo, hi = bounds[k]
                    a = lo[b * 2 + h] + j * 128
                    e = min(a + 128, hi[b * 2 + h])
                    core_chunks[k].append((a, max(a, e)))
            chunks_w.append(ch)
        struct['nch'].append(nch_w)
        struct['chunks'].append(chunks_w)
        struct['ngrp'].append(ngrp_w)

    ncht = sum(sum(x) for x in struct['nch'])
    ngt = sum(sum(x) for x in struct['ngrp'])
    struct['ncht'] = ncht
    struct['ngt'] = ngt

    # per-core blobs
    cores = []
    for k in range(NC):
        key, r, p, c, v = per_core[k]
        ind = np.zeros((ncht, 128, SEGW), np.float32)
        g1src = np.full((ncht, 128), -1, np.int64)
        pos_chunk = np.full((ncht, 128), -1, np.int64)   # remapped gather pos
        ci = 0
        for wi, (w0, wsz) in enumerate(WINDOWS):
            for h in (0, 1):
                for seg0 in struct['chunks'][wi][h]:
                    a, e = core_chunks[k][ci]
                    n = e - a
                    if n:
                        ind[ci, np.arange(n), r[a:e] - w0 - seg0] = v[a:e]
                        g1src[ci, :n] = c[a:e]
                        pos_chunk[ci, :n] = p[a:e]
                    ci += 1
        # L2 gather idx blob: half-major op order; padding -1 (trailing
        # skipped) except in the first GP_BUFS ops (uninitialized SBUF).
        idx_blocks = []
        op_i = 0
        for h in (0, 1):
            for wi in range(NW):
                base0 = sum(sum(struct['nch'][x]) for x in range(wi))
                cbase = base0 + (struct['nch'][wi][0] if h else 0)
                nch = struct['nch'][wi][h]
                ngr = struct['ngrp'][wi][h]
                for g in range(ngr):
                    blk = np.full(GIDX, -1, np.int64)
                    for j in range(GRP):
                        cj = g * GRP + j
                        if cj < nch:
                            blk[j * 128:(j + 1) * 128] = pos_chunk[cbase + cj]
                    blk[blk < 0] = 0
                    idx_blocks.append(_wrap_idx(blk))
                    op_i += 1
        g1 = np.zeros((ncht, 128, 128), BF16)
        cores.append({
            'idx': np.concatenate(idx_blocks, 1),
            'ind': np.ascontiguousarray(
                ind.transpose(1, 0, 2).reshape(128, ncht * SEGW)).astype(BF16),
            'g1src': g1src,
        })
    return struct, cores


def _build_nc(struct):
    ncht, ngt = struct['ncht'], struct['ngt']
    maxch_w = max(sum(x) for x in struct['nch'])

    nc = bacc.Bacc("TRN2", target_bir_lowering=False, debug=False,
                   enable_asserts=True, num_devices=NC, num_swdge_queues=4)
    f32, bf, i16 = mybir.dt.float32, mybir.dt.bfloat16, mybir.dt.int16
    AF = mybir.ActivationFunctionType

    g1_d = nc.dram_tensor("g1", [128, ncht * 128], bf, kind="ExternalInput")
    ind_d = nc.dram_tensor("ind", [128, ncht * SEGW], bf, kind="ExternalInput")
    idx_d = nc.dram_tensor("idx", [128, ngt * GIW], i16, kind="ExternalInput")
    w1_d = nc.dram_tensor("w1", [D, D], bf, kind="ExternalInput")
    w2_d = nc.dram_tensor("w2", [D, D], bf, kind="ExternalInput")
    w_d = nc.dram_tensor("w", [D, D], f32, kind="ExternalInput")
    bn_d = nc.dram_tensor("bn", [D, 8], f32, kind="ExternalInput")
    et_d = nc.dram_tensor("et", [128, NSH], f32, kind="ExternalInput")
    ebh_d = nc.dram_tensor("ebh", [128, B], f32, kind="ExternalInput")
    rgt_d = nc.dram_tensor("rgt", [128, B], f32, kind="ExternalInput")
    bidx_d = nc.dram_tensor("bidx", [128, 64], i16, kind="ExternalInput")
    idb_d = nc.dram_tensor("idb", [D, D], bf, kind="ExternalInput")
    idf_d = nc.dram_tensor("idf", [D, D], f32, kind="ExternalInput")
    out_d = nc.dram_tensor("out", [128, NW * 8 * 512], bf,
                           kind="ExternalOutput")

    from contextlib import ExitStack
    with tile.TileContext(nc) as tc:
        with ExitStack() as stack:
            pools = {}
            for nm, bufs, space in [
                    ("const", 1, None), ("g1p", 2, None), ("gp", GP_BUFS, None),
                    ("indp", 2, None), ("idxp", 1, None), ("sp", 1, None),
                    ("h1p", 2, None), ("hep", 2, None), ("h2p", 2, None),
                    ("h2bp", 2, None), ("etp", 2, None), ("fetp", 2, None),
                    ("fsp", 3, None), ("bp", 1, None), ("obp", 2, None),
                    ("pch", 2, "PSUM"), ("px", 2, "PSUM"), ("pt", 2, "PSUM"),
                    ("psc", 2, "PSUM"), ("dram", 1, "DRAM")]:
                kw = {"space": space} if space else {}
                pools[nm] = stack.enter_context(
                    tc.tile_pool(name=nm, bufs=bufs, **kw))
            cp, g1p, gp = pools["const"], pools["g1p"], pools["gp"]
            indp, idxp, spool = pools["indp"], pools["idxp"], pools["sp"]
            h1p, hep, h2p = pools["h1p"], pools["hep"], pools["h2p"]
            h2bp, etp, fetp = pools["h2bp"], pools["etp"], pools["fetp"]
            fsp, bp, obp = pools["fsp"], pools["bp"], pools["obp"]
            pch, px, pt = pools["pch"], pools["px"], pools["pt"]
            psc, dp = pools["psc"], pools["dram"]

            w1_t = cp.tile([D, D], bf); nc.scalar.dma_start(w1_t[:], w1_d[:])
            w2_t = cp.tile([D, D], bf); nc.scalar.dma_start(w2_t[:], w2_d[:])
            w_t = cp.tile([D, D], f32); nc.scalar.dma_start(w_t[:], w_d[:])
            bn_t = cp.tile([D, 8], f32); nc.scalar.dma_start(bn_t[:], bn_d[:])
            idb_t = cp.tile([D, D], bf); nc.scalar.dma_start(idb_t[:], idb_d[:])
            idf_t = cp.tile([D, D], f32); nc.scalar.dma_start(idf_t[:], idf_d[:])
            ebh_t = cp.tile([128, B], f32); nc.scalar.dma_start(ebh_t[:], ebh_d[:])
            rgt_t = cp.tile([128, B], f32); nc.scalar.dma_start(rgt_t[:], rgt_d[:])
            bidx_t = cp.tile([128, 64], i16); nc.scalar.dma_start(bidx_t[:], bidx_d[:])
            zl_t = cp.tile([1, 128], bf); nc.vector.memset(zl_t[:], 0.0)
            zr_t = cp.tile([1, 512], bf); nc.vector.memset(zr_t[:], 0.0)

            idx_all = idxp.tile([128, ngt * GIW], i16)
            nc.scalar.dma_start(idx_all[:], idx_d[:])
            s_t = spool.tile([128, NSH], bf, tag="s")

            hsh1 = dp.tile([NSH, D], bf, tag="hsh1")
            hsh2 = dp.tile([NSH, D], bf, tag="hsh2")
            hfa_sh = dp.tile([HALF, D], bf, tag="hfa_sh", addr_space="Shared")
            hfb = dp.tile([NB_ROWS, D], bf, tag="hfb", addr_space="Shared")
            fet_d = dp.tile([128, NSH], f32, tag="fet")

            def cbase_of(wi, h):
                base = sum(sum(struct['nch'][x]) for x in range(wi))
                return base + (struct['nch'][wi][0] if h else 0)

            def store_ht(src_bf, dst_dram, w0, wsz, layer):
                # PE-transpose [dim, ent] window into entity-major DRAM rows
                hent = hep.tile([128, 512], bf, tag="hent")
                for t in range(wsz // 128):
                    tp = pt.tile([128, 128], bf, tag="tp",
                                 name=f"tp{layer}_{w0}_{t}")
                    nc.tensor.transpose(tp[:], src_bf[:, t * 128:(t + 1) * 128],
                                        idb_t[:])
                    nc.vector.tensor_copy(hent[:, t * 128:(t + 1) * 128], tp[:])
                nt = wsz // 128
                nc.sync.dma_start(
                    dst_dram[w0:w0 + wsz, :].rearrange(
                        "(t p) d -> p t d", p=128),
                    hent[:, :wsz].rearrange("p (t d) -> p t d", t=nt))

            # ---------- layer 1: stream pre-gathered windows ----------
            for wi, (w0, wsz) in enumerate(WINDOWS):
                nch = sum(struct['nch'][wi])
                cbase = cbase_of(wi, 0)
                g1w = g1p.tile([128, maxch_w * 128], bf, tag="g1w")
                nc.sync.dma_start(
                    g1w[:, :nch * 128],
                    g1_d[:, cbase * 128:(cbase + nch) * 128])
                ind_t = indp.tile([128, maxch_w * SEGW], bf, tag="ind",
                                  name=f"ind1_{wi}")
                nc.scalar.dma_start(
                    ind_t[:, :nch * SEGW],
                    ind_d[:, cbase * SEGW:(cbase + nch) * SEGW])
                ps = pch.tile([128, 512], f32, tag="ps")
                nc.tensor.matmul(ps[:, :wsz], zl_t[:], zr_t[:, :wsz],
                                 start=True, stop=False, skip_group_check=True)
                allch = struct['chunks'][wi][0] + struct['chunks'][wi][1]
                for cj, seg0 in enumerate(allch):
                    nc.tensor.matmul(
                        ps[:, seg0:seg0 + SEGW],
                        g1w[:, cj * 128:(cj + 1) * 128],
                        ind_t[:, cj * SEGW:(cj + 1) * SEGW],
                        start=False, stop=(cj == nch - 1),
                        skip_group_check=True)
                nc.vector.tensor_copy(s_t[:, w0:w0 + wsz], ps[:, :wsz])
                # xform + h1 store
                xp = px.tile([128, 512], f32, tag="xp", name=f"xp1_{wi}")
                nc.tensor.matmul(xp[:, :wsz], w1_t[:], s_t[:, w0:w0 + wsz],
                                 start=True, stop=True)
                h1s = h1p.tile([128, 512], bf, tag="h1s")
                nc.scalar.activation(h1s[:, :wsz], xp[:, :wsz],
                                     AF.Relu, bias=bn_t[:, 0:1], scale=1.0)
                store_ht(h1s, hsh1, w0, wsz, 1)
                if wi == 7:
                    nc.gpsimd.collective_compute(
                        "AllGather", mybir.AluOpType.bypass,
                        replica_groups=[list(range(NC))],
                        ins=[hsh1[0:4096].opt()],
                        outs=[hfa_sh[:].opt()])
                elif wi == 12:
                    nc.gpsimd.collective_compute(
                        "AllGather", mybir.AluOpType.bypass,
                        replica_groups=[list(range(NC))],
                        ins=[hsh1[4096:NSH].opt()],
                        outs=[hfb[:].opt()])

            # ---------- layer 2: gather h1 from AG tables, half-major ----
            qn = [0]

            def gather_ops(wi, h, gbase):
                ngr = struct['ngrp'][wi][h]
                src = hfa_sh if h == 0 else hfb
                gts = []
                for g in range(ngr):
                    g_t = gp.tile([128, GIDX], bf, tag="g",
                                  name=f"g2_w{wi}_h{h}_{g}")
                    nc.gpsimd.dma_gather(
                        g_t[:].rearrange("p (c e) -> p c e", e=D),
                        src[:], idx_all[:, (gbase + g) * GIW:
                                        (gbase + g + 1) * GIW],
                        GIDX, GIDX, D, queue_num=qn[0] % 4)
                    qn[0] += 1
                    gts.append(g_t)
                return gts

            gbase = 0
            for h in (0, 1):
                for wi, (w0, wsz) in enumerate(WINDOWS):
                    nch = struct['nch'][wi][h]
                    cbase = cbase_of(wi, h)
                    if h == 1:
                        et_t = etp.tile([128, 512], f32, tag="et")
                        nc.scalar.dma_start(et_t[:, :wsz],
                                            et_d[:, w0:w0 + wsz])
                    gts = gather_ops(wi, h, gbase)
                    gbase += struct['ngrp'][wi][h]
                    ind_t = indp.tile([128, maxch_w * SEGW], bf, tag="ind",
                                      name=f"ind2_{wi}_{h}")
                    nc.scalar.dma_start(
                        ind_t[:, :nch * SEGW],
                        ind_d[:, cbase * SEGW:(cbase + nch) * SEGW])
                    ps = pch.tile([128, 512], f32, tag="ps")
                    nc.tensor.matmul(ps[:, :wsz], zl_t[:], zr_t[:, :wsz],
                                     start=True, stop=False,
                                     skip_group_check=True)
                    for cj, seg0 in enumerate(struct['chunks'][wi][h]):
                        nc.tensor.matmul(
                            ps[:, seg0:seg0 + SEGW],
                            gts[cj // GRP][:, (cj % GRP) * 128:
                                           (cj % GRP + 1) * 128],
                            ind_t[:, cj * SEGW:(cj + 1) * SEGW],
                            start=False, stop=(cj == nch - 1),
                            skip_group_check=True)
                    if h == 0:
                        nc.vector.tensor_copy(s_t[:, w0:w0 + wsz], ps[:, :wsz])
                    else:
                        nc.vector.tensor_tensor(
                            s_t[:, w0:w0 + wsz], s_t[:, w0:w0 + wsz],
                            ps[:, :wsz], mybir.AluOpType.add)
                        # tail: xform, fet accumulate, h2 store
                        xp = px.tile([128, 512], f32, tag="xp",
                                     name=f"xp2_{wi}")
                        nc.tensor.matmul(xp[:, :wsz], w2_t[:],
                                         s_t[:, w0:w0 + wsz],
                                         start=True, stop=True)
                        h2w = h2p.tile([128, 512], f32, tag="h2w")
                        nc.scalar.activation(h2w[:, :wsz], xp[:, :wsz],
                                             AF.Relu, bias=bn_t[:, 1:2],
                                             scale=1.0)
                        fet_t = fetp.tile([128, 512], f32, tag="fetw")
                        nc.vector.tensor_tensor(
                            fet_t[:, :wsz], et_t[:, :wsz], h2w[:, :wsz],
                            mybir.AluOpType.add)
                        nc.scalar.dma_start(fet_d[:, w0:w0 + wsz],
                                            fet_t[:, :wsz])
                        h2b = h2bp.tile([128, 512], bf, tag="h2b")
                        nc.vector.tensor_copy(h2b[:, :wsz], h2w[:, :wsz])
                        if wi == NW - 1:
                            nc.vector.memset(h2b[:, wsz - 1:wsz], 0.0)
                        store_ht(h2b, hsh2, w0, wsz, 2)

            # ---------- batch tail ----------
            tlo = bp.tile([128, B], bf)
            nc.gpsimd.dma_gather(
                tlo[:].rearrange("p (c e) -> p c e", e=D), hsh2[:],
                bidx_t[:, 0:64], 1024, 1024, D, queue_num=0)
            xpart = bp.tile([128, B], f32)
            nc.vector.tensor_copy(xpart[:], tlo[:])
            xin_dram = dp.tile([128, B], f32, tag="xin")
            xout_dram = dp.tile([128, B], f32, tag="xout", addr_space="Shared")
            nc.sync.dma_start(xin_dram[:], xpart[:])
            nc.gpsimd.collective_compute(
                "AllReduce", mybir.AluOpType.add,
                replica_groups=[list(range(NC))],
                ins=[xin_dram[:].opt()], outs=[xout_dram[:].opt()])
            xraw = bp.tile([128, B], f32)
            nc.sync.dma_start(xraw[:], xout_dram[:])
            nc.vector.tensor_tensor(xraw[:], xraw[:], ebh_t[:],
                                    mybir.AluOpType.add)
            xtb = bp.tile([128, B], f32)
            for j in range(8):
                tp = px.tile([128, 512], f32, tag="xp", name=f"bt_{j}")
                nc.tensor.transpose(tp[:, 0:128],
                                    xraw[:, j * 128:(j + 1) * 128], idf_t[:])
                nc.vector.tensor_scalar(
                    xtb[:, j * 128:(j + 1) * 128], tp[:, 0:128],
                    bn_t[:, 2:3], bn_t[:, 3:4],
                    mybir.AluOpType.mult, mybir.AluOpType.add)
            vmt = bp.tile([128, B], f32)
            for hb in range(2):
                sl = slice(hb * 512, hb * 512 + 512)
                wmp = px.tile([128, 512], f32, tag="xp", name=f"wm_{hb}")
                nc.tensor.matmul(wmp[:], w_t[:], rgt_t[:, sl],
                                 start=True, stop=True)
                nc.vector.tensor_tensor(vmt[:, sl], xtb[:, sl], wmp[:],
                                        mybir.AluOpType.mult)
            nc.vector.tensor_scalar(vmt[:], vmt[:], bn_t[:, 4:5], bn_t[:, 5:6],
                                    mybir.AluOpType.mult, mybir.AluOpType.add)

            # ---------- scoring ----------
            for wi, (w0, wsz) in enumerate(WINDOWS):
                fet_s = fsp.tile([128, 512], f32, tag="fets")
                nc.sync.dma_start(fet_s[:, :wsz], fet_d[:, w0:w0 + wsz])
                ob_w = obp.tile([128, 8 * 512], bf, tag="ob")
                for bt in range(8):
                    sc = psc.tile([128, 512], f32, tag="sc")
                    nc.tensor.matmul(sc[:, :wsz],
                                     vmt[:, bt * 128:(bt + 1) * 128],
                                     fet_s[:, :wsz],
                                     start=True, stop=True)
                    nc.scalar.activation(ob_w[:, bt * 512:bt * 512 + wsz],
                                         sc[:, :wsz], AF.Sigmoid)
                    if wsz < 512:
                        nc.vector.memset(
                            ob_w[:, bt * 512 + wsz:(bt + 1) * 512], 0.0)
                nc.sync.dma_start(out_d[:, wi * 4096:(wi + 1) * 4096], ob_w[:])
    nc.compile()
    return nc


def _host_prep(inputs):
    rows = np.asarray(inputs["adj_rows"]).astype(np.int64)
    cols = np.asarray(inputs["adj_cols"]).astype(np.int64)
    vals = np.asarray(inputs["adj_vals"], np.float32)
    E = np.asarray(inputs["E_emb"], np.float32)[np.asarray(inputs["init_ind"])]
    E_bf = E.astype(BF16)
    bh = np.asarray(inputs["batch_head"]).astype(np.int64)
    rel = np.asarray(inputs["batch_rel"]).astype(np.int64)
    R = np.asarray(inputs["R_emb"], np.float32)

    g0 = np.asarray(inputs["bn0_gamma"], np.float32) / np.sqrt(1.0 + BN_EPS)
    b0 = np.asarray(inputs["bn0_beta"], np.float32)
    g1 = np.asarray(inputs["bn1_gamma"], np.float32) / np.sqrt(1.0 + BN_EPS)
    b1v = np.asarray(inputs["bn1_beta"], np.float32)
    bn = np.ascontiguousarray(np.stack(
        [np.asarray(inputs["b1"], np.float32),
         np.asarray(inputs["b2"], np.float32),
         g0, b0, g1, b1v,
         np.zeros(D, np.float32), np.zeros(D, np.float32)], axis=1))

    bh_owner = bh // SH
    bh_local = bh - bh_owner * SH

    def slot_layout(a):          # [1024, D] -> [128, 8*D], slot i=(p,j)->j*128+p
        return np.ascontiguousarray(
            a.reshape(8, 128, D).transpose(1, 0, 2).reshape(128, 8 * D))

    ebh_l = slot_layout(E[bh])
    rgt = np.ascontiguousarray(R[rel].T.astype(np.float32))

    struct, cores = _make_plan(rows, cols, vals)
    ncht = struct['ncht']

    in_maps = []
    for k in range(NC):
        pl = cores[k]
        # L1 feature blob: [128 lanes, ncht, 128 dims], zeros on padding
        srcc = pl['g1src']
        g1 = np.zeros((ncht, 128, 128), BF16)
        m = srcc >= 0
        g1[m] = E_bf[srcc[m]]
        g1 = np.ascontiguousarray(
            g1.transpose(1, 0, 2).reshape(128, ncht * 128))
        et = np.zeros((D, NSH), np.float32)
        et[:, :SH] = E[k * SH:(k + 1) * SH].T
        in_maps.append({
            "g1": g1,
            "ind": pl['ind'],
            "idx": pl['idx'],
            "w1": np.asarray(inputs["W1"], np.float32).astype(BF16),
            "w2": np.asarray(inputs["W2"], np.float32).astype(BF16),
            "w": np.asarray(inputs["W"], np.float32),
            "bn": bn, "et": et, "ebh": ebh_l, "rgt": rgt,
            "bidx": _wrap_idx(np.where(bh_owner == k, bh_local, NSH - 1)),
            "idb": np.eye(D, dtype=np.float32).astype(BF16),
            "idf": np.eye(D, dtype=np.float32),
        })
    return struct, in_maps


def _run(inputs, trace=False):
    struct, in_maps = _host_prep(inputs)
    nc = _build_nc(struct)
    res = run_bass_kernel_spmd(nc, in_maps, core_ids=list(range(NC)),
                               trace=trace)
    outs = []
    for k in range(NC):
        o = res.results[k]["out"]            # [128, NW*8*512] bf16
        o = o.reshape(128, NW, 8, 512).transpose(2, 0, 1, 3)  # [bt, p, wi, c]
        o = o.reshape(B, NW * 512)[:, :SH]
        outs.append(o)
    return np.concatenate(outs, axis=1).astype(np.float32), res


def kernel(**inputs):
    out, _ = _run(inputs, trace=False)
    return out


# revision 15
# speedup vs baseline: 1.4326x; 1.2949x over previous
"""Trainium2 Bass kernel for nn_AblatedModel_40802189312754 (2-layer GNN + scoring).

Sharding: entities row-sharded 8 ways (6250/core, padded to 6400); batch
replicated; final [B, N] logits column-sharded by entity shard.

v2 design (descriptor-count optimized):
- L1 edge features are host-pre-gathered into a PARTITION-MAJOR blob
  [128, ncht*128] so each per-window load is one DMA with one large
  descriptor per partition (~390 GB/s instead of ~110).
- SpMM chunks are 128 edges x 64 segments (SEGW=64) to cut gather rows.
- L2 gathers read the AllGathered bf16 h1 table directly from the Shared
  collective-output tiles (no staging copies); the h0 table is one
  [32768, D] Shared tile filled by two AllGathers.
- All collective triggers are issued on gpsimd BEFORE the gather stream so
  the gather pipeline never blocks on a mid-stream collective wait.
- h1/h2 entity-major DRAM tables are produced via PE transposes (frees the
  Sync engine's HWDGE ring from descriptor-heavy DMA_TRANSPOSEs).
- Scoring output assembled per window [128, 8*512] and stored with one DMA
  per window into a partition-major out blob.
- All 8 cores share one instruction stream: per-(window,half) chunk counts
  are the max over cores; gather padding uses trailing -1 (skipped) where
  safe.
"""
import sys
sys.path.insert(0, '/opt/trn_rl_repo')

import numpy as np
import ml_dtypes

import concourse.bacc as bacc
import concourse.tile as tile
import concourse.mybir as mybir
from concourse.bass_utils import run_bass_kernel_spmd

BF16 = ml_dtypes.bfloat16

N_ENT = 50000
D = 128
B = 1024
NC = 8
SH = 6250            # real entities per shard
NSH = 6400           # padded shard size
BN_EPS = 1e-5
SEGW = 64            # bin width in segments
WINDOWS = [(w, min(512, NSH - w)) for w in range(0, NSH, 512)]  # 13 windows
NW = len(WINDOWS)
NBIN = NSH // SEGW   # 100 bins
GRP = 4              # chunks per L2 gather op
GIDX = GRP * 128     # idxs per gather op
GIW = GIDX // 16     # idx cols per gather op
HALF = 32768
NB_ROWS = NC * 2304  # 18432 (half-1 table rows)
GP_BUFS = 12         # L2 gather tile pool depth


def _remap(vid):
    """virtual id -> (half, pos within half-table) after AG reorder.

    h0 table = one AllGather of each core's hsh1[0:4096] (L1 windows 0-7);
    h1 table = one AllGather of hsh1[4096:6400] (windows 8-12).
    """
    k = vid // NSH
    local = vid - k * NSH
    w = local // 512
    pos = np.where(w <= 7, k * 4096 + local,
                   HALF + k * 2304 + (local - 4096))
    return (pos >= HALF).astype(np.int64), np.where(pos >= HALF, pos - HALF, pos)


def _wrap_idx(ids):
    """[n] -> [128, n//16] int16 gather-index layout (wrapped, replicated 8x)."""
    n = len(ids)
    w = ids.reshape(n // 16, 16).T
    return np.ascontiguousarray(np.tile(w, (8, 1)).astype(np.int16))


def _vid(ent):
    owner = ent // SH
    return owner * NSH + (ent - owner * SH)


def _make_plan(rows, cols, vals):
    """Uniform cross-core plan.

    Chunk order: window-major, within a window half 0 chunks then half 1
    (matches L1 streaming and the ind blob).  L2 processes half-major (all
    h0 windows, then all h1 windows) but indexes the same chunk ranges.

    Returns (struct, cores):
      struct['nch'][w][h]   = chunks for (window w, half h)
      struct['chunks'][w][h] = [seg0] per chunk (window-local segment base)
      struct['ngrp'][w][h]  = gather ops (GRP chunks each) for L2
      cores[k] = {'idx': [128, ngt*GIW] i16 (L2 ops, half-major),
                  'ind': [128, ncht*SEGW] bf16 (chunk-major),
                  'g1':  [128, ncht*128] bf16 (lane-major L1 features)}
    """
    vcol_all = _vid(cols)
    half_all, pos_all = _remap(vcol_all)
    per_core = []
    for k in range(NC):
        m = (rows >= k * SH) & (rows < (k + 1) * SH)
        r = rows[m] - k * SH
        h = half_all[m]
        p = pos_all[m]
        c = cols[m]
        v = vals[m].astype(np.float32)
        key = (r // SEGW) * 2 + h
        o = np.lexsort((p, r, key))
        per_core.append((key[o], r[o], p[o], c[o], v[o]))

    nch_bin = np.zeros((NBIN, 2), np.int64)
    bounds = []
    for k in range(NC):
        key = per_core[k][0]
        lo = np.searchsorted(key, np.arange(NBIN * 2))
        hi = np.searchsorted(key, np.arange(NBIN * 2) + 1)
        bounds.append((lo, hi))
        cnt = (hi - lo).reshape(NBIN, 2)
        nch_bin = np.maximum(nch_bin, -(-cnt // 128))

    struct = {'nch': [], 'chunks': [], 'ngrp': []}
    core_chunks = [[] for _ in range(NC)]   # per core: (a, b) edge ranges
    for wi, (w0, wsz) in enumerate(WINDOWS):
        b0, b1 = w0 // SEGW, (w0 + wsz) // SEGW
        nch_w, chunks_w, ngrp_w = [], [], []
        for h in (0, 1):
            cl = [(b, j) for b in range(b0, b1) for j in range(nch_bin[b, h])]
            nch_w.append(len(cl))
            ch = []
            for (b, j) in cl:
                ch.append((b - b0) * SEGW)
                for k in range(NC):
                    lo, hi = bounds[k]
                    a = lo[b * 2 + h] + j * 128
                    e = min(a + 128, hi[b * 2 + h])
                    core_chunks[k].append((a, max(a, e)))
            chunks_w.append(ch)
        struct['nch'].append(nch_w)
        struct['chunks'].append(chunks_w)

    ncht = sum(sum(x) for x in struct['nch'])
    # L2 gather ops: per table-half, packed across windows (GRP chunks/op)
    nch_half = [sum(struct['nch'][wi][h] for wi in range(NW)) for h in (0, 1)]
    struct['nops'] = [-(-n // GRP) for n in nch_half]
    ngt = sum(struct['nops'])
    struct['ncht'] = ncht
    struct['ngt'] = ngt

    # per-core blobs
    cores = []
    for k in range(NC):
        key, r, p, c, v = per_core[k]
        ind = np.zeros((ncht, 128, SEGW), np.float32)
        g1src = np.full((ncht, 128), -1, np.int64)
        pos_chunk = np.full((ncht, 128), -1, np.int64)   # remapped gather pos
        ci = 0
        for wi, (w0, wsz) in enumerate(WINDOWS):
            for h in (0, 1):
                for seg0 in struct['chunks'][wi][h]:
                    a, e = core_chunks[k][ci]
                    n = e - a
                    if n:
                        ind[ci, np.arange(n), r[a:e] - w0 - seg0] = v[a:e]
                        g1src[ci, :n] = c[a:e]
                        pos_chunk[ci, :n] = p[a:e]
                    ci += 1
        # L2 gather idx blob: per half, ops packed across window boundaries
        idx_blocks = []
        for h in (0, 1):
            pos_list = []
            for wi in range(NW):
                base0 = sum(sum(struct['nch'][x]) for x in range(wi))
                cbase = base0 + (struct['nch'][wi][0] if h else 0)
                for cj in range(struct['nch'][wi][h]):
                    pos_list.append(pos_chunk[cbase + cj])
            for g in range(struct['nops'][h]):
                blk = np.zeros(GIDX, np.int64)
                for j in range(GRP):
                    ci = g * GRP + j
                    if ci < len(pos_list):
                        p_ = pos_list[ci].copy()
                        p_[p_ < 0] = 0
                        blk[j * 128:(j + 1) * 128] = p_
                idx_blocks.append(_wrap_idx(blk))
        g1 = np.zeros((ncht, 128, 128), BF16)
        cores.append({
            'idx': np.concatenate(idx_blocks, 1),
            'ind': np.ascontiguousarray(
                ind.transpose(1, 0, 2).reshape(128, ncht * SEGW)).astype(BF16),
            'g1src': g1src,
        })
    return struct, cores


def _build_nc(struct):
    ncht, ngt = struct['ncht'], struct['ngt']
    maxch_w = max(sum(x) for x in struct['nch'])

    nc = bacc.Bacc("TRN2", target_bir_lowering=False, debug=False,
                   enable_asserts=True, num_devices=NC, num_swdge_queues=4)
    f32, bf, i16 = mybir.dt.float32, mybir.dt.bfloat16, mybir.dt.int16
    AF = mybir.ActivationFunctionType

    g1_d = nc.dram_tensor("g1", [128, ncht * 128], bf, kind="ExternalInput")
    ind_d = nc.dram_tensor("ind", [128, ncht * SEGW], bf, kind="ExternalInput")
    idx_d = nc.dram_tensor("idx", [128, ngt * GIW], i16, kind="ExternalInput")
    w1_d = nc.dram_tensor("w1", [D, D], bf, kind="ExternalInput")
    w2_d = nc.dram_tensor("w2", [D, D], bf, kind="ExternalInput")
    w_d = nc.dram_tensor("w", [D, D], f32, kind="ExternalInput")
    bn_d = nc.dram_tensor("bn", [D, 8], f32, kind="ExternalInput")
    et_d = nc.dram_tensor("et", [128, NSH], f32, kind="ExternalInput")
    ebh_d = nc.dram_tensor("ebh", [128, B], f32, kind="ExternalInput")
    rgt_d = nc.dram_tensor("rgt", [128, B], f32, kind="ExternalInput")
    bidx_d = nc.dram_tensor("bidx", [128, 64], i16, kind="ExternalInput")
    idb_d = nc.dram_tensor("idb", [D, D], bf, kind="ExternalInput")
    idf_d = nc.dram_tensor("idf", [D, D], f32, kind="ExternalInput")
    out_d = nc.dram_tensor("out", [128, NW * 8 * 512], bf,
                           kind="ExternalOutput")

    from contextlib import ExitStack
    with tile.TileContext(nc) as tc:
        with ExitStack() as stack:
            pools = {}
            for nm, bufs, space in [
                    ("const", 1, None), ("g1p", 2, None), ("gp", GP_BUFS, None),
                    ("indp", 2, None), ("idxp", 1, None), ("sp", 1, None),
                    ("h1p", 2, None), ("hep", 2, None), ("h2p", 2, None),
                    ("h2bp", 2, None), ("etp", 2, None), ("fetp", 2, None),
                    ("fsp", 3, None), ("bp", 1, None), ("obp", 2, None),
                    ("pch", 2, "PSUM"), ("px", 2, "PSUM"), ("pt", 2, "PSUM"),
                    ("psc", 2, "PSUM"), ("dram", 1, "DRAM")]:
                kw = {"space": space} if space else {}
                pools[nm] = stack.enter_context(
                    tc.tile_pool(name=nm, bufs=bufs, **kw))
            cp, g1p, gp = pools["const"], pools["g1p"], pools["gp"]
            indp, idxp, spool = pools["indp"], pools["idxp"], pools["sp"]
            h1p, hep, h2p = pools["h1p"], pools["hep"], pools["h2p"]
            h2bp, etp, fetp = pools["h2bp"], pools["etp"], pools["fetp"]
            fsp, bp, obp = pools["fsp"], pools["bp"], pools["obp"]
            pch, px, pt = pools["pch"], pools["px"], pools["pt"]
            psc, dp = pools["psc"], pools["dram"]

            w1_t = cp.tile([D, D], bf); nc.scalar.dma_start(w1_t[:], w1_d[:])
            w2_t = cp.tile([D, D], bf); nc.scalar.dma_start(w2_t[:], w2_d[:])
            w_t = cp.tile([D, D], f32); nc.scalar.dma_start(w_t[:], w_d[:])
            bn_t = cp.tile([D, 8], f32); nc.scalar.dma_start(bn_t[:], bn_d[:])
            idb_t = cp.tile([D, D], bf); nc.scalar.dma_start(idb_t[:], idb_d[:])
            idf_t = cp.tile([D, D], f32); nc.scalar.dma_start(idf_t[:], idf_d[:])
            ebh_t = cp.tile([128, B], f32); nc.scalar.dma_start(ebh_t[:], ebh_d[:])
            rgt_t = cp.tile([128, B], f32); nc.scalar.dma_start(rgt_t[:], rgt_d[:])
            bidx_t = cp.tile([128, 64], i16); nc.scalar.dma_start(bidx_t[:], bidx_d[:])
            zl_t = cp.tile([1, 128], bf); nc.vector.memset(zl_t[:], 0.0)
            zr_t = cp.tile([1, 512], bf); nc.vector.memset(zr_t[:], 0.0)

            idx_all = idxp.tile([128, ngt * GIW], i16)
            nc.scalar.dma_start(idx_all[:], idx_d[:])
            s_t = spool.tile([128, NSH], bf, tag="s")

            hsh1 = dp.tile([NSH, D], bf, tag="hsh1")
            hsh2 = dp.tile([NSH, D], bf, tag="hsh2")
            hfa_sh = dp.tile([HALF, D], bf, tag="hfa_sh", addr_space="Shared")
            hfb = dp.tile([NB_ROWS, D], bf, tag="hfb", addr_space="Shared")
            fet_d = dp.tile([128, NSH], f32, tag="fet")

            def cbase_of(wi, h):
                base = sum(sum(struct['nch'][x]) for x in range(wi))
                return base + (struct['nch'][wi][0] if h else 0)

            def store_ht(src_bf, dst_dram, w0, wsz, layer):
                # PE-transpose [dim, ent] window into entity-major DRAM rows
                hent = hep.tile([128, 512], bf, tag="hent")
                for t in range(wsz // 128):
                    tp = pt.tile([128, 128], bf, tag="tp",
                                 name=f"tp{layer}_{w0}_{t}")
                    nc.tensor.transpose(tp[:], src_bf[:, t * 128:(t + 1) * 128],
                                        idb_t[:])
                    nc.vector.tensor_copy(hent[:, t * 128:(t + 1) * 128], tp[:])
                nt = wsz // 128
                nc.sync.dma_start(
                    dst_dram[w0:w0 + wsz, :].rearrange(
                        "(t p) d -> p t d", p=128),
                    hent[:, :wsz].rearrange("p (t d) -> p t d", t=nt))

            # ---------- layer 1: stream pre-gathered windows ----------
            for wi, (w0, wsz) in enumerate(WINDOWS):
                nch = sum(struct['nch'][wi])
                cbase = cbase_of(wi, 0)
                g1w = g1p.tile([128, maxch_w * 128], bf, tag="g1w")
                nc.sync.dma_start(
                    g1w[:, :nch * 128],
                    g1_d[:, cbase * 128:(cbase + nch) * 128])
                ind_t = indp.tile([128, maxch_w * SEGW], bf, tag="ind",
                                  name=f"ind1_{wi}")
                nc.scalar.dma_start(
                    ind_t[:, :nch * SEGW],
                    ind_d[:, cbase * SEGW:(cbase + nch) * SEGW])
                ps = pch.tile([128, 512], f32, tag="ps")
                nc.tensor.matmul(ps[:, :wsz], zl_t[:], zr_t[:, :wsz],
                                 start=True, stop=False, skip_group_check=True)
                allch = struct['chunks'][wi][0] + struct['chunks'][wi][1]
                for cj, seg0 in enumerate(allch):
                    nc.tensor.matmul(
                        ps[:, seg0:seg0 + SEGW],
                        g1w[:, cj * 128:(cj + 1) * 128],
                        ind_t[:, cj * SEGW:(cj + 1) * SEGW],
                        start=False, stop=(cj == nch - 1),
                        skip_group_check=True)
                nc.vector.tensor_copy(s_t[:, w0:w0 + wsz], ps[:, :wsz])
                # xform + h1 store
                xp = px.tile([128, 512], f32, tag="xp", name=f"xp1_{wi}")
                nc.tensor.matmul(xp[:, :wsz], w1_t[:], s_t[:, w0:w0 + wsz],
                                 start=True, stop=True)
                h1s = h1p.tile([128, 512], bf, tag="h1s")
                nc.scalar.activation(h1s[:, :wsz], xp[:, :wsz],
                                     AF.Relu, bias=bn_t[:, 0:1], scale=1.0)
                store_ht(h1s, hsh1, w0, wsz, 1)
                if wi == 7:
                    nc.gpsimd.collective_compute(
                        "AllGather", mybir.AluOpType.bypass,
                        replica_groups=[list(range(NC))],
                        ins=[hsh1[0:4096].opt()],
                        outs=[hfa_sh[:].opt()])
                elif wi == 12:
                    nc.gpsimd.collective_compute(
                        "AllGather", mybir.AluOpType.bypass,
                        replica_groups=[list(range(NC))],
                        ins=[hsh1[4096:NSH].opt()],
                        outs=[hfb[:].opt()])

            # ---------- layer 2: gather h1 from AG tables, half-major ----
            qn = [0]
            op_base = [0, struct['nops'][0]]
            next_op = [0, 0]       # half-local next op to issue
            gts = {}               # global op index -> gather tile

            def ensure_ops(h, need):
                src = hfa_sh if h == 0 else hfb
                while next_op[h] < min(need, struct['nops'][h]):
                    g = op_base[h] + next_op[h]
                    g_t = gp.tile([128, GIDX], bf, tag="g",
                                  name=f"g2_{g}")
                    nc.gpsimd.dma_gather(
                        g_t[:].rearrange("p (c e) -> p c e", e=D),
                        src[:], idx_all[:, g * GIW:(g + 1) * GIW],
                        GIDX, GIDX, D, queue_num=qn[0] % 3 + 1)
                    qn[0] += 1
                    gts[g] = g_t
                    next_op[h] += 1

            for h in (0, 1):
                ch_cum = 0
                for wi, (w0, wsz) in enumerate(WINDOWS):
                    nch = struct['nch'][wi][h]
                    cbase = cbase_of(wi, h)
                    if h == 1:
                        et_t = etp.tile([128, 512], f32, tag="et")
                        nc.scalar.dma_start(et_t[:, :wsz],
                                            et_d[:, w0:w0 + wsz])
                    ensure_ops(h, -(-(ch_cum + nch) // GRP))
                    ind_t = indp.tile([128, maxch_w * SEGW], bf, tag="ind",
                                      name=f"ind2_{wi}_{h}")
                    nc.scalar.dma_start(
                        ind_t[:, :nch * SEGW],
                        ind_d[:, cbase * SEGW:(cbase + nch) * SEGW])
                    ps = pch.tile([128, 512], f32, tag="ps")
                    nc.tensor.matmul(ps[:, :wsz], zl_t[:], zr_t[:, :wsz],
                                     start=True, stop=False,
                                     skip_group_check=True)
                    for cj, seg0 in enumerate(struct['chunks'][wi][h]):
                        ch_i = ch_cum + cj
                        nc.tensor.matmul(
                            ps[:, seg0:seg0 + SEGW],
                            gts[op_base[h] + ch_i // GRP][
                                :, (ch_i % GRP) * 128:
                                (ch_i % GRP + 1) * 128],
                            ind_t[:, cj * SEGW:(cj + 1) * SEGW],
                            start=False, stop=(cj == nch - 1),
                            skip_group_check=True)
                    ch_cum += nch
                    if h == 0:
                        nc.vector.tensor_copy(s_t[:, w0:w0 + wsz], ps[:, :wsz])
                    else:
                        nc.vector.tensor_tensor(
                            s_t[:, w0:w0 + wsz], s_t[:, w0:w0 + wsz],
                            ps[:, :wsz], mybir.AluOpType.add)
                        # tail: xform, fet accumulate, h2 store
                        xp = px.tile([128, 512], f32, tag="xp",
                                     name=f"xp2_{wi}")
                        nc.tensor.matmul(xp[:, :wsz], w2_t[:],
                                         s_t[:, w0:w0 + wsz],
                                         start=True, stop=True)
                        h2w = h2p.tile([128, 512], f32, tag="h2w")
                        nc.scalar.activation(h2w[:, :wsz], xp[:, :wsz],
                                             AF.Relu, bias=bn_t[:, 1:2],
                                             scale=1.0)
                        fet_t = fetp.tile([128, 512], f32, tag="fetw")
                        nc.vector.tensor_tensor(
                            fet_t[:, :wsz], et_t[:, :wsz], h2w[:, :wsz],
                            mybir.AluOpType.add)
                        nc.scalar.dma_start(fet_d[:, w0:w0 + wsz],
                                            fet_t[:, :wsz])
                        h2b = h2bp.tile([128, 512], bf, tag="h2b")
                        nc.vector.tensor_copy(h2b[:, :wsz], h2w[:, :wsz])
                        if wi == NW - 1:
                            nc.vector.memset(h2b[:, wsz - 1:wsz], 0.0)
                        store_ht(h2b, hsh2, w0, wsz, 2)

            # ---------- batch tail ----------
            tlo = bp.tile([128, B], bf)
            nc.gpsimd.dma_gather(
                tlo[:].rearrange("p (c e) -> p c e", e=D), hsh2[:],
                bidx_t[:, 0:64], 1024, 1024, D, queue_num=0)
            xin_dram = dp.tile([128, B], bf, tag="xin")
            xout_dram = dp.tile([128, B], bf, tag="xout", addr_space="Shared")
            nc.sync.dma_start(xin_dram[:], tlo[:])
            # bf16 AllReduce is exact here: each slot has one real
            # contributor (its owner); the other 7 cores add the zeroed
            # pad row NSH-1.
            nc.gpsimd.collective_compute(
                "AllReduce", mybir.AluOpType.add,
                replica_groups=[list(range(NC))],
                ins=[xin_dram[:].opt()], outs=[xout_dram[:].opt()])
            xag = bp.tile([128, B], bf)
            nc.sync.dma_start(xag[:], xout_dram[:])
            xraw = bp.tile([128, B], f32)
            nc.vector.tensor_tensor(xraw[:], xag[:], ebh_t[:],
                                    mybir.AluOpType.add)
            xtb = bp.tile([128, B], f32)
            for j in range(8):
                tp = px.tile([128, 512], f32, tag="xp", name=f"bt_{j}")
                nc.tensor.transpose(tp[:, 0:128],
                                    xraw[:, j * 128:(j + 1) * 128], idf_t[:])
                nc.vector.tensor_scalar(
                    xtb[:, j * 128:(j + 1) * 128], tp[:, 0:128],
                    bn_t[:, 2:3], bn_t[:, 3:4],
                    mybir.AluOpType.mult, mybir.AluOpType.add)
            vmt = bp.tile([128, B], f32)
            for hb in range(2):
                sl = slice(hb * 512, hb * 512 + 512)
                wmp = px.tile([128, 512], f32, tag="xp", name=f"wm_{hb}")
                nc.tensor.matmul(wmp[:], w_t[:], rgt_t[:, sl],
                                 start=True, stop=True)
                nc.vector.tensor_tensor(vmt[:, sl], xtb[:, sl], wmp[:],
                                        mybir.AluOpType.mult)
            nc.vector.tensor_scalar(vmt[:], vmt[:], bn_t[:, 4:5], bn_t[:, 5:6],
                                    mybir.AluOpType.mult, mybir.AluOpType.add)

            # ---------- scoring ----------
            for wi, (w0, wsz) in enumerate(WINDOWS):
                fet_s = fsp.tile([128, 512], f32, tag="fets")
                nc.sync.dma_start(fet_s[:, :wsz], fet_d[:, w0:w0 + wsz])
                ob_w = obp.tile([128, 8 * 512], bf, tag="ob")
                for bt in range(8):
                    sc = psc.tile([128, 512], f32, tag="sc")
                    nc.tensor.matmul(sc[:, :wsz],
                                     vmt[:, bt * 128:(bt + 1) * 128],
                                     fet_s[:, :wsz],
                                     start=True, stop=True)
                    nc.scalar.activation(ob_w[:, bt * 512:bt * 512 + wsz],
                                         sc[:, :wsz], AF.Sigmoid)
                    if wsz < 512:
                        nc.vector.memset(
                            ob_w[:, bt * 512 + wsz:(bt + 1) * 512], 0.0)
                nc.sync.dma_start(out_d[:, wi * 4096:(wi + 1) * 4096], ob_w[:])
    nc.compile()
    return nc


def _host_prep(inputs):
    rows = np.asarray(inputs["adj_rows"]).astype(np.int64)
    cols = np.asarray(inputs["adj_cols"]).astype(np.int64)
    vals = np.asarray(inputs["adj_vals"], np.float32)
    E = np.asarray(inputs["E_emb"], np.float32)[np.asarray(inputs["init_ind"])]
    E_bf = E.astype(BF16)
    bh = np.asarray(inputs["batch_head"]).astype(np.int64)
    rel = np.asarray(inputs["batch_rel"]).astype(np.int64)
    R = np.asarray(inputs["R_emb"], np.float32)

    g0 = np.asarray(inputs["bn0_gamma"], np.float32) / np.sqrt(1.0 + BN_EPS)
    b0 = np.asarray(inputs["bn0_beta"], np.float32)
    g1 = np.asarray(inputs["bn1_gamma"], np.float32) / np.sqrt(1.0 + BN_EPS)
    b1v = np.asarray(inputs["bn1_beta"], np.float32)
    bn = np.ascontiguousarray(np.stack(
        [np.asarray(inputs["b1"], np.float32),
         np.asarray(inputs["b2"], np.float32),
         g0, b0, g1, b1v,
         np.zeros(D, np.float32), np.zeros(D, np.float32)], axis=1))

    bh_owner = bh // SH
    bh_local = bh - bh_owner * SH

    def slot_layout(a):          # [1024, D] -> [128, 8*D], slot i=(p,j)->j*128+p
        return np.ascontiguousarray(
            a.reshape(8, 128, D).transpose(1, 0, 2).reshape(128, 8 * D))

    ebh_l = slot_layout(E[bh])
    rgt = np.ascontiguousarray(R[rel].T.astype(np.float32))

    struct, cores = _make_plan(rows, cols, vals)
    ncht = struct['ncht']

    in_maps = []
    for k in range(NC):
        pl = cores[k]
        # L1 feature blob: [128 lanes, ncht, 128 dims], zeros on padding
        srcc = pl['g1src']
        g1 = np.zeros((ncht, 128, 128), BF16)
        m = srcc >= 0
        g1[m] = E_bf[srcc[m]]
        g1 = np.ascontiguousarray(
            g1.transpose(1, 0, 2).reshape(128, ncht * 128))
        et = np.zeros((D, NSH), np.float32)
        et[:, :SH] = E[k * SH:(k + 1) * SH].T
        in_maps.append({
            "g1": g1,
            "ind": pl['ind'],
            "idx": pl['idx'],
            "w1": np.asarray(inputs["W1"], np.float32).astype(BF16),
            "w2": np.asarray(inputs["W2"], np.float32).astype(BF16),
            "w": np.asarray(inputs["W"], np.float32),
            "bn": bn, "et": et, "ebh": ebh_l, "rgt": rgt,
            "bidx": _wrap_idx(np.where(bh_owner == k, bh_local, NSH - 1)),
            "idb": np.eye(D, dtype=np.float32).astype(BF16),
            "idf": np.eye(D, dtype=np.float32),
        })
    return struct, in_maps


def _run(inputs, trace=False):
    struct, in_maps = _host_prep(inputs)
    nc = _build_nc(struct)
    res = run_bass_kernel_spmd(nc, in_maps, core_ids=list(range(NC)),
                               trace=trace)
    outs = []
    for k in range(NC):
        o = res.results[k]["out"]            # [128, NW*8*512] bf16
        o = o.reshape(128, NW, 8, 512).transpose(2, 0, 1, 3)  # [bt, p, wi, c]
        o = o.reshape(B, NW * 512)[:, :SH]
        outs.append(o)
    return np.concatenate(outs, axis=1).astype(np.float32), res


def kernel(**inputs):
    out, _ = _run(inputs, trace=False)
    return out


# revision 26
# speedup vs baseline: 1.9785x; 1.3811x over previous
"""Trainium2 Bass kernel for nn_AblatedModel_40802189312754 (2-layer GNN + scoring).

Sharding: entities row-sharded 8 ways (6250/core, padded to 6400); batch
replicated; final [B, N] logits column-sharded by entity shard.

v2 design (descriptor-count optimized):
- L1 edge features are host-pre-gathered into a PARTITION-MAJOR blob
  [128, ncht*128] so each per-window load is one DMA with one large
  descriptor per partition (~390 GB/s instead of ~110).
- SpMM chunks are 128 edges x 64 segments (SEGW=64) to cut gather rows.
- L2 gathers read the AllGathered bf16 h1 table directly from the Shared
  collective-output tiles (no staging copies); the h0 table is one
  [32768, D] Shared tile filled by two AllGathers.
- All collective triggers are issued on gpsimd BEFORE the gather stream so
  the gather pipeline never blocks on a mid-stream collective wait.
- h1/h2 entity-major DRAM tables are produced via PE transposes (frees the
  Sync engine's HWDGE ring from descriptor-heavy DMA_TRANSPOSEs).
- Scoring output assembled per window [128, 8*512] and stored with one DMA
  per window into a partition-major out blob.
- All 8 cores share one instruction stream: per-(window,half) chunk counts
  are the max over cores; gather padding uses trailing -1 (skipped) where
  safe.
"""
import sys
sys.path.insert(0, '/opt/trn_rl_repo')

import numpy as np
import ml_dtypes

import concourse.bacc as bacc
import concourse.tile as tile
import concourse.mybir as mybir
from concourse.bass_utils import run_bass_kernel_spmd

BF16 = ml_dtypes.bfloat16

N_ENT = 50000
D = 128
B = 1024
NC = 8
SH = 6250            # real entities per shard
NSH = 6400           # padded shard size
BN_EPS = 1e-5
SEGW = 64            # bin width in segments
WINDOWS = [(w, min(512, NSH - w)) for w in range(0, NSH, 512)]  # 13 windows
NW = len(WINDOWS)
NBIN = NSH // SEGW   # 100 bins
GRP = 4              # chunks per L2 gather op
GIDX = GRP * 128     # idxs per gather op
GIW = GIDX // 16     # idx cols per gather op
HALF = 32768
NB_ROWS = NC * 2304  # 18432 (half-1 table rows)
GP_BUFS = 12         # L2 gather tile pool depth


def _loc2slot(local):
    """local entity id -> DRAM row in the (p,t)-interleaved h tables.

    Each window's 512 rows are stored partition-major: entity w0+t*128+p
    lands at row w0 + p*nt + t (nt = wsz//128), so the per-window store
    writes nt*256B contiguous per partition instead of 256B.
    """
    local = np.asarray(local)
    w = local // 512
    ww = local - 512 * w
    nt = np.where(w < 12, 4, 2)
    return w * 512 + (ww % 128) * nt + ww // 128


def _remap(vid):
    """virtual id -> (half, pos within half-table) after AG reorder.

    h0 table = one AllGather of each core's hsh1[0:4096] (L1 windows 0-7);
    h1 table = one AllGather of hsh1[4096:6400] (windows 8-12).
    """
    k = vid // NSH
    local = vid - k * NSH
    slot = _loc2slot(local)
    pos = np.where(slot < 4096, k * 4096 + slot,
                   HALF + k * 2304 + (slot - 4096))
    return (pos >= HALF).astype(np.int64), np.where(pos >= HALF, pos - HALF, pos)


def _wrap_idx(ids):
    """[n] -> [128, n//16] int16 gather-index layout (wrapped, replicated 8x)."""
    n = len(ids)
    w = ids.reshape(n // 16, 16).T
    return np.ascontiguousarray(np.tile(w, (8, 1)).astype(np.int16))


def _vid(ent):
    owner = ent // SH
    return owner * NSH + (ent - owner * SH)


def _make_plan(rows, cols, vals):
    """Uniform cross-core plan.

    Chunk order: window-major, within a window half 0 chunks then half 1
    (matches L1 streaming and the ind blob).  L2 processes half-major (all
    h0 windows, then all h1 windows) but indexes the same chunk ranges.

    Returns (struct, cores):
      struct['nch'][w][h]   = chunks for (window w, half h)
      struct['chunks'][w][h] = [seg0] per chunk (window-local segment base)
      struct['ngrp'][w][h]  = gather ops (GRP chunks each) for L2
      cores[k] = {'idx': [128, ngt*GIW] i16 (L2 ops, half-major),
                  'ind': [128, ncht*SEGW] bf16 (chunk-major),
                  'g1':  [128, ncht*128] bf16 (lane-major L1 features)}
    """
    vcol_all = _vid(cols)
    half_all, pos_all = _remap(vcol_all)
    per_core = []
    for k in range(NC):
        m = (rows >= k * SH) & (rows < (k + 1) * SH)
        r = rows[m] - k * SH
        h = half_all[m]
        p = pos_all[m]
        c = cols[m]
        v = vals[m].astype(np.float32)
        key = (r // 512) * 2 + h
        o = np.lexsort((p, r, key))
        per_core.append((key[o], r[o], p[o], c[o], v[o]))

    bounds = []
    for k in range(NC):
        key = per_core[k][0]
        lo = np.searchsorted(key, np.arange(NW * 2))
        hi = np.searchsorted(key, np.arange(NW * 2) + 1)
        bounds.append((lo, hi))

    # Span chunks: per (window, half), consecutive 128-edge chunks of each
    # core's row-sorted edge list; the shared PSUM span of chunk c is the
    # min/max row range over cores.
    struct = {'nch': [], 'chunks': []}
    core_chunks = [[] for _ in range(NC)]   # per core: (a, b) edge ranges
    coff = 0
    for wi, (w0, wsz) in enumerate(WINDOWS):
        nch_w, chunks_w = [], []
        for h in (0, 1):
            ky = wi * 2 + h
            cnts = [bounds[k][1][ky] - bounds[k][0][ky] for k in range(NC)]
            nch = -(-max(cnts) // 128)
            nch_w.append(nch)
            ch = []
            for cidx in range(nch):
                s0, e0 = wsz, 0
                for k in range(NC):
                    lo, hi = bounds[k][0][ky], bounds[k][1][ky]
                    a = lo + cidx * 128
                    e = min(a + 128, hi)
                    core_chunks[k].append((a, max(a, e)))
                    if a < hi:
                        rr = per_core[k][1]
                        s0 = min(s0, rr[a] - w0)
                        e0 = max(e0, rr[e - 1] - w0 + 1)
                seg0 = int(s0)
                n = min(-(-(int(e0) - seg0) // 8) * 8, wsz - seg0)
                ch.append((seg0, n, coff))
                coff += n
            chunks_w.append(ch)
        struct['nch'].append(nch_w)
        struct['chunks'].append(chunks_w)
    struct['indcols'] = coff

    ncht = sum(sum(x) for x in struct['nch'])
    # L2 gather ops: per table-half, packed across windows (GRP chunks/op)
    nch_half = [sum(struct['nch'][wi][h] for wi in range(NW)) for h in (0, 1)]
    struct['nops'] = [-(-n // GRP) for n in nch_half]
    ngt = sum(struct['nops'])
    struct['ncht'] = ncht
    struct['ngt'] = ngt

    # per-core blobs
    cores = []
    for k in range(NC):
        key, r, p, c, v = per_core[k]
        ind = np.zeros((128, struct['indcols']), np.float32)
        g1src = np.full((ncht, 128), -1, np.int64)
        pos_chunk = np.full((ncht, 128), -1, np.int64)   # remapped gather pos
        ci = 0
        for wi, (w0, wsz) in enumerate(WINDOWS):
            for h in (0, 1):
                for (seg0, nn, coff) in struct['chunks'][wi][h]:
                    a, e = core_chunks[k][ci]
                    n = e - a
                    if n:
                        ind[np.arange(n), coff + r[a:e] - w0 - seg0] = v[a:e]
                        g1src[ci, :n] = c[a:e]
                        pos_chunk[ci, :n] = p[a:e]
                    ci += 1
        # L2 gather idx blob: per half, ops packed across window boundaries
        idx_blocks = []
        for h in (0, 1):
            pos_list = []
            for wi in range(NW):
                base0 = sum(sum(struct['nch'][x]) for x in range(wi))
                cbase = base0 + (struct['nch'][wi][0] if h else 0)
                for cj in range(struct['nch'][wi][h]):
                    pos_list.append(pos_chunk[cbase + cj])
            for g in range(struct['nops'][h]):
                blk = np.zeros(GIDX, np.int64)
                for j in range(GRP):
                    ci = g * GRP + j
                    if ci < len(pos_list):
                        p_ = pos_list[ci].copy()
                        p_[p_ < 0] = 0
                        blk[j * 128:(j + 1) * 128] = p_
                idx_blocks.append(_wrap_idx(blk))
        g1 = np.zeros((ncht, 128, 128), BF16)
        cores.append({
            'idx': np.concatenate(idx_blocks, 1),
            'ind': np.ascontiguousarray(ind).astype(BF16),
            'g1src': g1src,
        })
    return struct, cores


def _build_nc(struct):
    ncht, ngt = struct['ncht'], struct['ngt']
    maxch_w = max(sum(x) for x in struct['nch'])

    def wspan(wi, h=None):
        """(first ind col, n ind cols) for window wi (both halves or one)."""
        chs = (struct['chunks'][wi][0] + struct['chunks'][wi][1]
               if h is None else struct['chunks'][wi][h])
        c0 = chs[0][2]
        return c0, chs[-1][2] + chs[-1][1] - c0

    maxic_w = max(wspan(wi)[1] for wi in range(NW))

    nc = bacc.Bacc("TRN2", target_bir_lowering=False, debug=False,
                   enable_asserts=True, num_devices=NC, num_swdge_queues=4)
    f32, bf, i16 = mybir.dt.float32, mybir.dt.bfloat16, mybir.dt.int16
    AF = mybir.ActivationFunctionType

    g1_d = nc.dram_tensor("g1", [128, ncht * 128], bf, kind="ExternalInput")
    ind_d = nc.dram_tensor("ind", [128, struct['indcols']], bf,
                           kind="ExternalInput")
    idx_d = nc.dram_tensor("idx", [128, ngt * GIW], i16, kind="ExternalInput")
    w1_d = nc.dram_tensor("w1", [D, D], bf, kind="ExternalInput")
    w2_d = nc.dram_tensor("w2", [D, D], bf, kind="ExternalInput")
    w_d = nc.dram_tensor("w", [D, D], f32, kind="ExternalInput")
    bn_d = nc.dram_tensor("bn", [D, 8], f32, kind="ExternalInput")
    et_d = nc.dram_tensor("et", [128, NSH], f32, kind="ExternalInput")
    ebh_d = nc.dram_tensor("ebh", [128, B], f32, kind="ExternalInput")
    rgt_d = nc.dram_tensor("rgt", [128, B], f32, kind="ExternalInput")
    bidx_d = nc.dram_tensor("bidx", [128, 64], i16, kind="ExternalInput")
    idb_d = nc.dram_tensor("idb", [D, D], bf, kind="ExternalInput")
    idf_d = nc.dram_tensor("idf", [D, D], f32, kind="ExternalInput")
    out_d = nc.dram_tensor("out", [128, NW * 8 * 512], bf,
                           kind="ExternalOutput")

    from contextlib import ExitStack
    with tile.TileContext(nc) as tc:
        with ExitStack() as stack:
            pools = {}
            for nm, bufs, space in [
                    ("const", 1, None), ("g1p", 2, None), ("gp", GP_BUFS, None),
                    ("indp", 2, None), ("idxp", 1, None), ("sp", 1, None),
                    ("h1p", 2, None), ("hep", 2, None), ("h2p", 2, None),
                    ("h2bp", 2, None), ("etp", 2, None), ("fetp", 2, None),
                    ("fsp", 3, None), ("bp", 1, None), ("obp", 2, None),
                    ("pch", 2, "PSUM"), ("px", 2, "PSUM"), ("pt", 2, "PSUM"),
                    ("psc", 2, "PSUM"), ("dram", 1, "DRAM")]:
                kw = {"space": space} if space else {}
                pools[nm] = stack.enter_context(
                    tc.tile_pool(name=nm, bufs=bufs, **kw))
            cp, g1p, gp = pools["const"], pools["g1p"], pools["gp"]
            indp, idxp, spool = pools["indp"], pools["idxp"], pools["sp"]
            h1p, hep, h2p = pools["h1p"], pools["hep"], pools["h2p"]
            h2bp, etp, fetp = pools["h2bp"], pools["etp"], pools["fetp"]
            fsp, bp, obp = pools["fsp"], pools["bp"], pools["obp"]
            pch, px, pt = pools["pch"], pools["px"], pools["pt"]
            psc, dp = pools["psc"], pools["dram"]

            w1_t = cp.tile([D, D], bf); nc.scalar.dma_start(w1_t[:], w1_d[:])
            w2_t = cp.tile([D, D], bf); nc.scalar.dma_start(w2_t[:], w2_d[:])
            w_t = cp.tile([D, D], f32); nc.scalar.dma_start(w_t[:], w_d[:])
            bn_t = cp.tile([D, 8], f32); nc.scalar.dma_start(bn_t[:], bn_d[:])
            idb_t = cp.tile([D, D], bf); nc.scalar.dma_start(idb_t[:], idb_d[:])
            idf_t = cp.tile([D, D], f32); nc.scalar.dma_start(idf_t[:], idf_d[:])
            ebh_t = cp.tile([128, B], f32); nc.scalar.dma_start(ebh_t[:], ebh_d[:])
            rgt_t = cp.tile([128, B], f32); nc.scalar.dma_start(rgt_t[:], rgt_d[:])
            bidx_t = cp.tile([128, 64], i16); nc.scalar.dma_start(bidx_t[:], bidx_d[:])
            zl_t = cp.tile([1, 128], bf); nc.vector.memset(zl_t[:], 0.0)
            zr_t = cp.tile([1, 512], bf); nc.vector.memset(zr_t[:], 0.0)

            idx_all = idxp.tile([128, ngt * GIW], i16)
            nc.scalar.dma_start(idx_all[:], idx_d[:])
            s_t = spool.tile([128, NSH], bf, tag="s")

            hsh1 = dp.tile([NSH, D], bf, tag="hsh1")
            hsh2 = dp.tile([NSH, D], bf, tag="hsh2")
            hfa_sh = dp.tile([HALF, D], bf, tag="hfa_sh", addr_space="Shared")
            hfb = dp.tile([NB_ROWS, D], bf, tag="hfb", addr_space="Shared")
            fet_d = dp.tile([128, NSH], f32, tag="fet")

            def cbase_of(wi, h):
                base = sum(sum(struct['nch'][x]) for x in range(wi))
                return base + (struct['nch'][wi][0] if h else 0)

            def store_ht(src_bf, dst_dram, w0, wsz, layer):
                # PE-transpose [dim, ent] window into (p,t)-interleaved
                # entity rows: DRAM row w0 + p*nt + t holds entity w0+t*128+p,
                # so each partition writes nt*256B contiguous.
                hent = hep.tile([128, 512], bf, tag="hent")
                nt = wsz // 128
                for t in range(nt):
                    tp = pt.tile([128, 128], bf, tag="tp",
                                 name=f"tp{layer}_{w0}_{t}")
                    nc.tensor.transpose(tp[:], src_bf[:, t * 128:(t + 1) * 128],
                                        idb_t[:])
                    nc.vector.tensor_copy(hent[:, t * 128:(t + 1) * 128], tp[:])
                nc.sync.dma_start(
                    dst_dram[w0:w0 + wsz, :].rearrange(
                        "(p t) d -> p t d", t=nt),
                    hent[:, :wsz].rearrange("p (t d) -> p t d", t=nt))

            # ---------- layer 1: stream pre-gathered windows ----------
            for wi, (w0, wsz) in enumerate(WINDOWS):
                nch = sum(struct['nch'][wi])
                cbase = cbase_of(wi, 0)
                ic0, icn = wspan(wi)
                g1w = g1p.tile([128, maxch_w * 128], bf, tag="g1w")
                nc.sync.dma_start(
                    g1w[:, :nch * 128],
                    g1_d[:, cbase * 128:(cbase + nch) * 128])
                ind_t = indp.tile([128, maxic_w], bf, tag="ind",
                                  name=f"ind1_{wi}")
                nc.scalar.dma_start(ind_t[:, :icn], ind_d[:, ic0:ic0 + icn])
                ps = pch.tile([128, 512], f32, tag="ps")
                nc.tensor.matmul(ps[:, :wsz], zl_t[:], zr_t[:, :wsz],
                                 start=True, stop=False, skip_group_check=True)
                allch = struct['chunks'][wi][0] + struct['chunks'][wi][1]
                for cj, (seg0, nn, coff) in enumerate(allch):
                    nc.tensor.matmul(
                        ps[:, seg0:seg0 + nn],
                        g1w[:, cj * 128:(cj + 1) * 128],
                        ind_t[:, coff - ic0:coff - ic0 + nn],
                        start=False, stop=(cj == nch - 1),
                        skip_group_check=True)
                nc.scalar.activation(s_t[:, w0:w0 + wsz], ps[:, :wsz],
                                     AF.Copy)
                # xform + h1 store
                xp = px.tile([128, 512], f32, tag="xp", name=f"xp1_{wi}")
                nc.tensor.matmul(xp[:, :wsz], w1_t[:], s_t[:, w0:w0 + wsz],
                                 start=True, stop=True)
                h1s = h1p.tile([128, 512], bf, tag="h1s")
                nc.scalar.activation(h1s[:, :wsz], xp[:, :wsz],
                                     AF.Relu, bias=bn_t[:, 0:1], scale=1.0)
                store_ht(h1s, hsh1, w0, wsz, 1)
                if wi == 7:
                    nc.gpsimd.collective_compute(
                        "AllGather", mybir.AluOpType.bypass,
                        replica_groups=[list(range(NC))],
                        ins=[hsh1[0:4096].opt()],
                        outs=[hfa_sh[:].opt()])
                elif wi == 12:
                    nc.gpsimd.collective_compute(
                        "AllGather", mybir.AluOpType.bypass,
                        replica_groups=[list(range(NC))],
                        ins=[hsh1[4096:NSH].opt()],
                        outs=[hfb[:].opt()])

            # ---------- layer 2: gather h1 from AG tables, half-major ----
            qn = [0]
            op_base = [0, struct['nops'][0]]
            next_op = [0, 0]       # half-local next op to issue
            gts = {}               # global op index -> gather tile

            def ensure_ops(h, need):
                src = hfa_sh if h == 0 else hfb
                while next_op[h] < min(need, struct['nops'][h]):
                    g = op_base[h] + next_op[h]
                    g_t = gp.tile([128, GIDX], bf, tag="g",
                                  name=f"g2_{g}")
                    nc.gpsimd.dma_gather(
                        g_t[:].rearrange("p (c e) -> p c e", e=D),
                        src[:], idx_all[:, g * GIW:(g + 1) * GIW],
                        GIDX, GIDX, D, queue_num=qn[0] % 3 + 1)
                    qn[0] += 1
                    gts[g] = g_t
                    next_op[h] += 1

            for h in (0, 1):
                ch_cum = 0
                for wi, (w0, wsz) in enumerate(WINDOWS):
                    nch = struct['nch'][wi][h]
                    cbase = cbase_of(wi, h)
                    if h == 1:
                        et_t = etp.tile([128, 512], f32, tag="et")
                        nc.scalar.dma_start(et_t[:, :wsz],
                                            et_d[:, w0:w0 + wsz])
                    ensure_ops(h, -(-(ch_cum + nch) // GRP))
                    ic0, icn = wspan(wi, h)
                    ind_t = indp.tile([128, maxic_w], bf, tag="ind",
                                      name=f"ind2_{wi}_{h}")
                    nc.scalar.dma_start(ind_t[:, :icn],
                                        ind_d[:, ic0:ic0 + icn])
                    ps = pch.tile([128, 512], f32, tag="ps")
                    nc.tensor.matmul(ps[:, :wsz], zl_t[:], zr_t[:, :wsz],
                                     start=True, stop=False,
                                     skip_group_check=True)
                    for cj, (seg0, nn, coff) in enumerate(
                            struct['chunks'][wi][h]):
                        ch_i = ch_cum + cj
                        nc.tensor.matmul(
                            ps[:, seg0:seg0 + nn],
                            gts[op_base[h] + ch_i // GRP][
                                :, (ch_i % GRP) * 128:
                                (ch_i % GRP + 1) * 128],
                            ind_t[:, coff - ic0:coff - ic0 + nn],
                            start=False, stop=(cj == nch - 1),
                            skip_group_check=True)
                    ch_cum += nch
                    if h == 0:
                        nc.scalar.activation(s_t[:, w0:w0 + wsz], ps[:, :wsz],
                                             AF.Copy)
                    else:
                        nc.vector.tensor_tensor(
                            s_t[:, w0:w0 + wsz], s_t[:, w0:w0 + wsz],
                            ps[:, :wsz], mybir.AluOpType.add)
                        # tail: xform, fet accumulate, h2 store
                        xp = px.tile([128, 512], f32, tag="xp",
                                     name=f"xp2_{wi}")
                        nc.tensor.matmul(xp[:, :wsz], w2_t[:],
                                         s_t[:, w0:w0 + wsz],
                                         start=True, stop=True)
                        h2w = h2p.tile([128, 512], f32, tag="h2w")
                        nc.scalar.activation(h2w[:, :wsz], xp[:, :wsz],
                                             AF.Relu, bias=bn_t[:, 1:2],
                                             scale=1.0)
                        fet_t = fetp.tile([128, 512], f32, tag="fetw")
                        nc.vector.tensor_tensor(
                            fet_t[:, :wsz], et_t[:, :wsz], h2w[:, :wsz],
                            mybir.AluOpType.add)
                        nc.scalar.dma_start(fet_d[:, w0:w0 + wsz],
                                            fet_t[:, :wsz])
                        h2b = h2bp.tile([128, 512], bf, tag="h2b")
                        nc.vector.tensor_copy(h2b[:, :wsz], h2w[:, :wsz])
                        if wi == NW - 1:
                            nc.vector.memset(h2b[:, wsz - 1:wsz], 0.0)
                        store_ht(h2b, hsh2, w0, wsz, 2)

            # ---------- batch tail ----------
            tlo = bp.tile([128, B], bf)
            nc.gpsimd.dma_gather(
                tlo[:].rearrange("p (c e) -> p c e", e=D), hsh2[:],
                bidx_t[:, 0:64], 1024, 1024, D, queue_num=0)
            xin_dram = dp.tile([128, B], bf, tag="xin")
            xout_dram = dp.tile([128, B], bf, tag="xout", addr_space="Shared")
            nc.sync.dma_start(xin_dram[:], tlo[:])
            # bf16 AllReduce is exact here: each slot has one real
            # contributor (its owner); the other 7 cores add the zeroed
            # pad row NSH-1.
            nc.gpsimd.collective_compute(
                "AllReduce", mybir.AluOpType.add,
                replica_groups=[list(range(NC))],
                ins=[xin_dram[:].opt()], outs=[xout_dram[:].opt()])
            xag = bp.tile([128, B], bf)
            nc.sync.dma_start(xag[:], xout_dram[:])
            xraw = bp.tile([128, B], f32)
            nc.vector.tensor_tensor(xraw[:], xag[:], ebh_t[:],
                                    mybir.AluOpType.add)
            xtb = bp.tile([128, B], f32)
            for j in range(8):
                tp = px.tile([128, 512], f32, tag="xp", name=f"bt_{j}")
                nc.tensor.transpose(tp[:, 0:128],
                                    xraw[:, j * 128:(j + 1) * 128], idf_t[:])
                nc.vector.tensor_scalar(
                    xtb[:, j * 128:(j + 1) * 128], tp[:, 0:128],
                    bn_t[:, 2:3], bn_t[:, 3:4],
                    mybir.AluOpType.mult, mybir.AluOpType.add)
            vmt = bp.tile([128, B], f32)
            for hb in range(2):
                sl = slice(hb * 512, hb * 512 + 512)
                wmp = px.tile([128, 512], f32, tag="xp", name=f"wm_{hb}")
                nc.tensor.matmul(wmp[:], w_t[:], rgt_t[:, sl],
                                 start=True, stop=True)
                nc.vector.tensor_tensor(vmt[:, sl], xtb[:, sl], wmp[:],
                                        mybir.AluOpType.mult)
            nc.vector.tensor_scalar(vmt[:], vmt[:], bn_t[:, 4:5], bn_t[:, 5:6],
                                    mybir.AluOpType.mult, mybir.AluOpType.add)

            # ---------- scoring ----------
            for wi, (w0, wsz) in enumerate(WINDOWS):
                fet_s = fsp.tile([128, 512], f32, tag="fets")
                nc.sync.dma_start(fet_s[:, :wsz], fet_d[:, w0:w0 + wsz])
                ob_w = obp.tile([128, 8 * 512], bf, tag="ob")
                for bt in range(8):
                    sc = psc.tile([128, 512], f32, tag="sc")
                    nc.tensor.matmul(sc[:, :wsz],
                                     vmt[:, bt * 128:(bt + 1) * 128],
                                     fet_s[:, :wsz],
                                     start=True, stop=True)
                    nc.scalar.activation(ob_w[:, bt * 512:bt * 512 + wsz],
                                         sc[:, :wsz], AF.Sigmoid)
                    if wsz < 512:
                        nc.vector.memset(
                            ob_w[:, bt * 512 + wsz:(bt + 1) * 512], 0.0)
                nc.sync.dma_start(out_d[:, wi * 4096:(wi + 1) * 4096], ob_w[:])
    nc.compile()
    return nc


def _host_prep(inputs):
    rows = np.asarray(inputs["adj_rows"]).astype(np.int64)
    cols = np.asarray(inputs["adj_cols"]).astype(np.int64)
    vals = np.asarray(inputs["adj_vals"], np.float32)
    E = np.asarray(inputs["E_emb"], np.float32)[np.asarray(inputs["init_ind"])]
    E_bf = E.astype(BF16)
    bh = np.asarray(inputs["batch_head"]).astype(np.int64)
    rel = np.asarray(inputs["batch_rel"]).astype(np.int64)
    R = np.asarray(inputs["R_emb"], np.float32)

    g0 = np.asarray(inputs["bn0_gamma"], np.float32) / np.sqrt(1.0 + BN_EPS)
    b0 = np.asarray(inputs["bn0_beta"], np.float32)
    g1 = np.asarray(inputs["bn1_gamma"], np.float32) / np.sqrt(1.0 + BN_EPS)
    b1v = np.asarray(inputs["bn1_beta"], np.float32)
    bn = np.ascontiguousarray(np.stack(
        [np.asarray(inputs["b1"], np.float32),
         np.asarray(inputs["b2"], np.float32),
         g0, b0, g1, b1v,
         np.zeros(D, np.float32), np.zeros(D, np.float32)], axis=1))

    bh_owner = bh // SH
    bh_local = bh - bh_owner * SH

    def slot_layout(a):          # [1024, D] -> [128, 8*D], slot i=(p,j)->j*128+p
        return np.ascontiguousarray(
            a.reshape(8, 128, D).transpose(1, 0, 2).reshape(128, 8 * D))

    ebh_l = slot_layout(E[bh])
    rgt = np.ascontiguousarray(R[rel].T.astype(np.float32))

    struct, cores = _make_plan(rows, cols, vals)
    ncht = struct['ncht']

    in_maps = []
    for k in range(NC):
        pl = cores[k]
        # L1 feature blob: [128 lanes, ncht, 128 dims], zeros on padding
        srcc = pl['g1src']
        g1 = np.zeros((ncht, 128, 128), BF16)
        m = srcc >= 0
        g1[m] = E_bf[srcc[m]]
        g1 = np.ascontiguousarray(
            g1.transpose(1, 0, 2).reshape(128, ncht * 128))
        et = np.zeros((D, NSH), np.float32)
        et[:, :SH] = E[k * SH:(k + 1) * SH].T
        in_maps.append({
            "g1": g1,
            "ind": pl['ind'],
            "idx": pl['idx'],
            "w1": np.asarray(inputs["W1"], np.float32).astype(BF16),
            "w2": np.asarray(inputs["W2"], np.float32).astype(BF16),
            "w": np.asarray(inputs["W"], np.float32),
            "bn": bn, "et": et, "ebh": ebh_l, "rgt": rgt,
            "bidx": _wrap_idx(np.where(bh_owner == k,
                                       _loc2slot(bh_local), NSH - 1)),
            "idb": np.eye(D, dtype=np.float32).astype(BF16),
            "idf": np.eye(D, dtype=np.float32),
        })
    return struct, in_maps


def _run(inputs, trace=False):
    struct, in_maps = _host_prep(inputs)
    nc = _build_nc(struct)
    res = run_bass_kernel_spmd(nc, in_maps, core_ids=list(range(NC)),
                               trace=trace)
    outs = []
    for k in range(NC):
        o = res.results[k]["out"]            # [128, NW*8*512] bf16
        o = o.reshape(128, NW, 8, 512).transpose(2, 0, 1, 3)  # [bt, p, wi, c]
        o = o.reshape(B, NW * 512)[:, :SH]
        outs.append(o)
    return np.concatenate(outs, axis=1).astype(np.float32), res


def kernel(**inputs):
    out, _ = _run(inputs, trace=False)
    return out


# revision 42
# speedup vs baseline: 1.9967x; 1.0092x over previous
"""Trainium2 Bass kernel for nn_AblatedModel_40802189312754 (2-layer GNN + scoring).

Sharding: entities row-sharded 8 ways (6250/core, padded to 6400); batch
replicated; final [B, N] logits column-sharded by entity shard.

v2 design (descriptor-count optimized):
- L1 edge features are host-pre-gathered into a PARTITION-MAJOR blob
  [128, ncht*128] so each per-window load is one DMA with one large
  descriptor per partition (~390 GB/s instead of ~110).
- SpMM chunks are 128 edges x 64 segments (SEGW=64) to cut gather rows.
- L2 gathers read the AllGathered bf16 h1 table directly from the Shared
  collective-output tiles (no staging copies); the h0 table is one
  [32768, D] Shared tile filled by two AllGathers.
- All collective triggers are issued on gpsimd BEFORE the gather stream so
  the gather pipeline never blocks on a mid-stream collective wait.
- h1/h2 entity-major DRAM tables are produced via PE transposes (frees the
  Sync engine's HWDGE ring from descriptor-heavy DMA_TRANSPOSEs).
- Scoring output assembled per window [128, 8*512] and stored with one DMA
  per window into a partition-major out blob.
- All 8 cores share one instruction stream: per-(window,half) chunk counts
  are the max over cores; gather padding uses trailing -1 (skipped) where
  safe.
"""
import sys
sys.path.insert(0, '/opt/trn_rl_repo')

import numpy as np
import ml_dtypes

import concourse.bacc as bacc
import concourse.tile as tile
import concourse.mybir as mybir
from concourse.bass_utils import run_bass_kernel_spmd

BF16 = ml_dtypes.bfloat16

N_ENT = 50000
D = 128
B = 1024
NC = 8
SH = 6250            # real entities per shard
NSH = 6400           # padded shard size
BN_EPS = 1e-5
SEGW = 64            # bin width in segments
WINDOWS = [(w, min(512, NSH - w)) for w in range(0, NSH, 512)]  # 13 windows
NW = len(WINDOWS)
NBIN = NSH // SEGW   # 100 bins
GRP = 4              # chunks per L2 gather op
GIDX = GRP * 128     # idxs per gather op
GIW = GIDX // 16     # idx cols per gather op
NPIECE = 3           # AllGathered h1 table pieces (w0-3 / w4-7 / w8-12)
NB_ROWS = NC * 2304  # 18432 (piece-2 table rows)
GP_BUFS = 12         # L2 gather tile pool depth


def _loc2slot(local):
    """local entity id -> DRAM row in the (p,t)-interleaved h tables.

    Each window's 512 rows are stored partition-major: entity w0+t*128+p
    lands at row w0 + p*nt + t (nt = wsz//128), so the per-window store
    writes nt*256B contiguous per partition instead of 256B.
    """
    local = np.asarray(local)
    w = local // 512
    ww = local - 512 * w
    nt = np.where(w < 12, 4, 2)
    return w * 512 + (ww % 128) * nt + ww // 128


def _remap(vid):
    """virtual id -> (piece, pos within piece table) after AG reorder.

    Three AllGathered tables: piece 0 = L1 windows 0-3 (slots 0:2048/core),
    piece 1 = windows 4-7 (2048:4096), piece 2 = windows 8-12 (4096:6400).
    """
    k = vid // NSH
    local = vid - k * NSH
    slot = _loc2slot(local)
    piece = np.minimum(slot // 2048, 2)
    base = np.where(piece == 2, 4096, piece * 2048)
    rows = np.where(piece == 2, 2304, 2048)
    pos = k * rows + (slot - base)
    return piece, pos


def _wrap_idx(ids):
    """[n] -> [128, n//16] int16 gather-index layout (wrapped, replicated 8x)."""
    n = len(ids)
    w = ids.reshape(n // 16, 16).T
    return np.ascontiguousarray(np.tile(w, (8, 1)).astype(np.int16))


def _vid(ent):
    owner = ent // SH
    return owner * NSH + (ent - owner * SH)


def _make_plan(rows, cols, vals):
    """Uniform cross-core plan.

    Chunk order: window-major, within a window half 0 chunks then half 1
    (matches L1 streaming and the ind blob).  L2 processes half-major (all
    h0 windows, then all h1 windows) but indexes the same chunk ranges.

    Returns (struct, cores):
      struct['nch'][w][h]   = chunks for (window w, half h)
      struct['chunks'][w][h] = [seg0] per chunk (window-local segment base)
      struct['ngrp'][w][h]  = gather ops (GRP chunks each) for L2
      cores[k] = {'idx': [128, ngt*GIW] i16 (L2 ops, half-major),
                  'ind': [128, ncht*SEGW] bf16 (chunk-major),
                  'g1':  [128, ncht*128] bf16 (lane-major L1 features)}
    """
    vcol_all = _vid(cols)
    piece_all, pos_all = _remap(vcol_all)
    per_core = []
    for k in range(NC):
        m = (rows >= k * SH) & (rows < (k + 1) * SH)
        r = rows[m] - k * SH
        h = piece_all[m]
        p = pos_all[m]
        c = cols[m]
        v = vals[m].astype(np.float32)
        key = (r // 512) * NPIECE + h
        o = np.lexsort((p, r, key))
        per_core.append((key[o], r[o], p[o], c[o], v[o]))

    bounds = []
    for k in range(NC):
        key = per_core[k][0]
        lo = np.searchsorted(key, np.arange(NW * NPIECE))
        hi = np.searchsorted(key, np.arange(NW * NPIECE) + 1)
        bounds.append((lo, hi))

    # Span chunks: per (window, piece), consecutive 128-edge chunks of each
    # core's row-sorted edge list; the shared PSUM span of chunk c is the
    # min/max row range over cores.
    struct = {'nch': [], 'chunks': []}
    core_chunks = [[] for _ in range(NC)]   # per core: (a, b) edge ranges
    coff = 0
    for wi, (w0, wsz) in enumerate(WINDOWS):
        nch_w, chunks_w = [], []
        for h in range(NPIECE):
            ky = wi * NPIECE + h
            cnts = [bounds[k][1][ky] - bounds[k][0][ky] for k in range(NC)]
            nch = -(-max(cnts) // 128)
            nch_w.append(nch)
            ch = []
            for cidx in range(nch):
                s0, e0 = wsz, 0
                for k in range(NC):
                    lo, hi = bounds[k][0][ky], bounds[k][1][ky]
                    a = lo + cidx * 128
                    e = min(a + 128, hi)
                    core_chunks[k].append((a, max(a, e)))
                    if a < hi:
                        rr = per_core[k][1]
                        s0 = min(s0, rr[a] - w0)
                        e0 = max(e0, rr[e - 1] - w0 + 1)
                seg0 = int(s0)
                n = min(-(-(int(e0) - seg0) // 8) * 8, wsz - seg0)
                ch.append((seg0, n, coff))
                coff += n
            chunks_w.append(ch)
        struct['nch'].append(nch_w)
        struct['chunks'].append(chunks_w)
    struct['indcols'] = coff

    ncht = sum(sum(x) for x in struct['nch'])
    # L2 gather ops: per table piece, packed across windows (GRP chunks/op)
    nch_p = [sum(struct['nch'][wi][h] for wi in range(NW))
             for h in range(NPIECE)]
    struct['nops'] = [-(-n // GRP) for n in nch_p]
    ngt = sum(struct['nops'])
    struct['ncht'] = ncht
    struct['ngt'] = ngt

    # per-core blobs
    cores = []
    for k in range(NC):
        key, r, p, c, v = per_core[k]
        ind = np.zeros((128, struct['indcols']), np.float32)
        g1src = np.full((ncht, 128), -1, np.int64)
        pos_chunk = np.full((ncht, 128), -1, np.int64)   # remapped gather pos
        ci = 0
        for wi, (w0, wsz) in enumerate(WINDOWS):
            for h in range(NPIECE):
                for (seg0, nn, coff) in struct['chunks'][wi][h]:
                    a, e = core_chunks[k][ci]
                    n = e - a
                    if n:
                        ind[np.arange(n), coff + r[a:e] - w0 - seg0] = v[a:e]
                        g1src[ci, :n] = c[a:e]
                        pos_chunk[ci, :n] = p[a:e]
                    ci += 1
        # L2 gather idx blob: per piece, ops packed across window boundaries
        idx_blocks = []
        for h in range(NPIECE):
            pos_list = []
            for wi in range(NW):
                base0 = sum(sum(struct['nch'][x]) for x in range(wi))
                cbase = base0 + sum(struct['nch'][wi][:h])
                for cj in range(struct['nch'][wi][h]):
                    pos_list.append(pos_chunk[cbase + cj])
            for g in range(struct['nops'][h]):
                blk = np.zeros(GIDX, np.int64)
                for j in range(GRP):
                    ci = g * GRP + j
                    if ci < len(pos_list):
                        p_ = pos_list[ci].copy()
                        p_[p_ < 0] = 0
                        blk[j * 128:(j + 1) * 128] = p_
                idx_blocks.append(_wrap_idx(blk))
        g1 = np.zeros((ncht, 128, 128), BF16)
        cores.append({
            'idx': np.concatenate(idx_blocks, 1),
            'ind': np.ascontiguousarray(ind).astype(BF16),
            'g1src': g1src,
        })
    return struct, cores


def _build_nc(struct):
    ncht, ngt = struct['ncht'], struct['ngt']
    maxch_w = max(sum(x) for x in struct['nch'])

    def wspan(wi, h=None):
        """(first ind col, n ind cols) for window wi (all pieces or one)."""
        chs = (sum((struct['chunks'][wi][p] for p in range(NPIECE)), [])
               if h is None else struct['chunks'][wi][h])
        c0 = chs[0][2]
        return c0, chs[-1][2] + chs[-1][1] - c0

    maxic_w = max(wspan(wi)[1] for wi in range(NW))

    nc = bacc.Bacc("TRN2", target_bir_lowering=False, debug=False,
                   enable_asserts=True, num_devices=NC, num_swdge_queues=4)
    f32, bf, i16 = mybir.dt.float32, mybir.dt.bfloat16, mybir.dt.int16
    AF = mybir.ActivationFunctionType

    g1_d = nc.dram_tensor("g1", [128, ncht * 128], bf, kind="ExternalInput")
    ind_d = nc.dram_tensor("ind", [128, struct['indcols']], bf,
                           kind="ExternalInput")
    idx_d = nc.dram_tensor("idx", [128, ngt * GIW], i16, kind="ExternalInput")
    w1_d = nc.dram_tensor("w1", [D, D], bf, kind="ExternalInput")
    w2_d = nc.dram_tensor("w2", [D, D], bf, kind="ExternalInput")
    w_d = nc.dram_tensor("w", [D, D], f32, kind="ExternalInput")
    bn_d = nc.dram_tensor("bn", [D, 8], f32, kind="ExternalInput")
    et_d = nc.dram_tensor("et", [128, NSH], f32, kind="ExternalInput")
    ebh_d = nc.dram_tensor("ebh", [128, B], f32, kind="ExternalInput")
    rgt_d = nc.dram_tensor("rgt", [128, B], f32, kind="ExternalInput")
    bidx_d = nc.dram_tensor("bidx", [128, 64], i16, kind="ExternalInput")
    idb_d = nc.dram_tensor("idb", [D, D], bf, kind="ExternalInput")
    idf_d = nc.dram_tensor("idf", [D, D], f32, kind="ExternalInput")
    out_d = nc.dram_tensor("out", [128, NW * 8 * 512], bf,
                           kind="ExternalOutput")

    from contextlib import ExitStack
    with tile.TileContext(nc) as tc:
        with ExitStack() as stack:
            pools = {}
            for nm, bufs, space in [
                    ("const", 1, None), ("g1p", 2, None), ("gp", GP_BUFS, None),
                    ("indp", 2, None), ("idxp", 1, None), ("sp", 1, None),
                    ("h1p", 2, None), ("hep", 2, None), ("h2p", 2, None),
                    ("h2bp", 2, None), ("etp", 2, None), ("fetp", 2, None),
                    ("fsp", 3, None), ("bp", 1, None), ("obp", 2, None),
                    ("pch", 2, "PSUM"), ("px", 2, "PSUM"), ("pt", 2, "PSUM"),
                    ("psc", 2, "PSUM"), ("dram", 1, "DRAM")]:
                kw = {"space": space} if space else {}
                pools[nm] = stack.enter_context(
                    tc.tile_pool(name=nm, bufs=bufs, **kw))
            cp, g1p, gp = pools["const"], pools["g1p"], pools["gp"]
            indp, idxp, spool = pools["indp"], pools["idxp"], pools["sp"]
            h1p, hep, h2p = pools["h1p"], pools["hep"], pools["h2p"]
            h2bp, etp, fetp = pools["h2bp"], pools["etp"], pools["fetp"]
            fsp, bp, obp = pools["fsp"], pools["bp"], pools["obp"]
            pch, px, pt = pools["pch"], pools["px"], pools["pt"]
            psc, dp = pools["psc"], pools["dram"]

            w1_t = cp.tile([D, D], bf); nc.scalar.dma_start(w1_t[:], w1_d[:])
            w2_t = cp.tile([D, D], bf); nc.scalar.dma_start(w2_t[:], w2_d[:])
            w_t = cp.tile([D, D], f32); nc.scalar.dma_start(w_t[:], w_d[:])
            bn_t = cp.tile([D, 8], f32); nc.scalar.dma_start(bn_t[:], bn_d[:])
            idb_t = cp.tile([D, D], bf); nc.scalar.dma_start(idb_t[:], idb_d[:])
            idf_t = cp.tile([D, D], f32); nc.scalar.dma_start(idf_t[:], idf_d[:])
            ebh_t = cp.tile([128, B], f32); nc.scalar.dma_start(ebh_t[:], ebh_d[:])
            rgt_t = cp.tile([128, B], f32); nc.scalar.dma_start(rgt_t[:], rgt_d[:])
            bidx_t = cp.tile([128, 64], i16); nc.scalar.dma_start(bidx_t[:], bidx_d[:])
            zl_t = cp.tile([1, 128], bf); nc.vector.memset(zl_t[:], 0.0)
            zr_t = cp.tile([1, 512], bf); nc.vector.memset(zr_t[:], 0.0)

            idx_all = idxp.tile([128, ngt * GIW], i16)
            nc.scalar.dma_start(idx_all[:], idx_d[:])
            s_t = spool.tile([128, NSH], bf, tag="s")

            hsh1 = dp.tile([NSH, D], bf, tag="hsh1")
            hsh2 = dp.tile([NSH, D], bf, tag="hsh2")
            hfa0 = dp.tile([16384, D], bf, tag="hfa0", addr_space="Shared")
            hfa1 = dp.tile([16384, D], bf, tag="hfa1", addr_space="Shared")
            hfb = dp.tile([NB_ROWS, D], bf, tag="hfb", addr_space="Shared")
            fet_d = dp.tile([128, NSH], f32, tag="fet")

            def cbase_of(wi, h):
                base = sum(sum(struct['nch'][x]) for x in range(wi))
                return base + sum(struct['nch'][wi][:h])

            engs = (nc.sync, nc.scalar)

            def store_ht(src_bf, dst_dram, w0, wsz, layer, eng=None):
                # PE-transpose [dim, ent] window into (p,t)-interleaved
                # entity rows: DRAM row w0 + p*nt + t holds entity w0+t*128+p,
                # so each partition writes nt*256B contiguous.
                hent = hep.tile([128, 512], bf, tag="hent")
                nt = wsz // 128
                for t in range(nt):
                    tp = pt.tile([128, 128], bf, tag="tp",
                                 name=f"tp{layer}_{w0}_{t}")
                    nc.tensor.transpose(tp[:], src_bf[:, t * 128:(t + 1) * 128],
                                        idb_t[:])
                    nc.vector.tensor_copy(hent[:, t * 128:(t + 1) * 128], tp[:])
                (eng or nc.sync).dma_start(
                    dst_dram[w0:w0 + wsz, :].rearrange(
                        "(p t) d -> p t d", t=nt),
                    hent[:, :wsz].rearrange("p (t d) -> p t d", t=nt))

            # ---------- layer 1: stream pre-gathered windows ----------
            for wi, (w0, wsz) in enumerate(WINDOWS):
                nch = sum(struct['nch'][wi])
                cbase = cbase_of(wi, 0)
                ic0, icn = wspan(wi)
                g1w = g1p.tile([128, maxch_w * 128], bf, tag="g1w")
                engs[wi % 2].dma_start(
                    g1w[:, :nch * 128],
                    g1_d[:, cbase * 128:(cbase + nch) * 128])
                ind_t = indp.tile([128, maxic_w], bf, tag="ind",
                                  name=f"ind1_{wi}")
                engs[(wi + 1) % 2].dma_start(ind_t[:, :icn],
                                             ind_d[:, ic0:ic0 + icn])
                ps = pch.tile([128, 512], f32, tag="ps")
                nc.tensor.matmul(ps[:, :wsz], zl_t[:], zr_t[:, :wsz],
                                 start=True, stop=False, skip_group_check=True)
                allch = sum((struct['chunks'][wi][p] for p in range(NPIECE)),
                            [])
                for cj, (seg0, nn, coff) in enumerate(allch):
                    nc.tensor.matmul(
                        ps[:, seg0:seg0 + nn],
                        g1w[:, cj * 128:(cj + 1) * 128],
                        ind_t[:, coff - ic0:coff - ic0 + nn],
                        start=False, stop=(cj == nch - 1),
                        skip_group_check=True)
                nc.scalar.activation(s_t[:, w0:w0 + wsz], ps[:, :wsz],
                                     AF.Copy)
                # xform + h1 store
                xp = px.tile([128, 512], f32, tag="xp", name=f"xp1_{wi}")
                nc.tensor.matmul(xp[:, :wsz], w1_t[:], s_t[:, w0:w0 + wsz],
                                 start=True, stop=True)
                h1s = h1p.tile([128, 512], bf, tag="h1s")
                nc.scalar.activation(h1s[:, :wsz], xp[:, :wsz],
                                     AF.Relu, bias=bn_t[:, 0:1], scale=1.0)
                store_ht(h1s, hsh1, w0, wsz, 1, eng=engs[(wi + 1) % 2])
                if wi == 3:
                    nc.gpsimd.collective_compute(
                        "AllGather", mybir.AluOpType.bypass,
                        replica_groups=[list(range(NC))],
                        ins=[hsh1[0:2048].opt()], outs=[hfa0[:].opt()])
                elif wi == 7:
                    nc.gpsimd.collective_compute(
                        "AllGather", mybir.AluOpType.bypass,
                        replica_groups=[list(range(NC))],
                        ins=[hsh1[2048:4096].opt()], outs=[hfa1[:].opt()])
                elif wi == 12:
                    nc.gpsimd.collective_compute(
                        "AllGather", mybir.AluOpType.bypass,
                        replica_groups=[list(range(NC))],
                        ins=[hsh1[4096:NSH].opt()], outs=[hfb[:].opt()])

            # ---------- layer 2: gather h1 from AG tables, piece-major ----
            qn = [0]
            op_base = [0, struct['nops'][0],
                       struct['nops'][0] + struct['nops'][1]]
            next_op = [0, 0, 0]    # piece-local next op to issue
            gts = {}               # global op index -> gather tile

            def ensure_ops(h, need):
                src = (hfa0, hfa1, hfb)[h]
                while next_op[h] < min(need, struct['nops'][h]):
                    g = op_base[h] + next_op[h]
                    g_t = gp.tile([128, GIDX], bf, tag="g",
                                  name=f"g2_{g}")
                    nc.gpsimd.dma_gather(
                        g_t[:].rearrange("p (c e) -> p c e", e=D),
                        src[:], idx_all[:, g * GIW:(g + 1) * GIW],
                        GIDX, GIDX, D, queue_num=qn[0] % 3 + 1)
                    qn[0] += 1
                    gts[g] = g_t
                    next_op[h] += 1

            for h in range(NPIECE):
                ch_cum = 0
                for wi, (w0, wsz) in enumerate(WINDOWS):
                    nch = struct['nch'][wi][h]
                    cbase = cbase_of(wi, h)
                    if h == NPIECE - 1:
                        et_t = etp.tile([128, 512], f32, tag="et")
                        nc.scalar.dma_start(et_t[:, :wsz],
                                            et_d[:, w0:w0 + wsz])
                    ensure_ops(h, -(-(ch_cum + nch) // GRP))
                    ic0, icn = wspan(wi, h)
                    ind_t = indp.tile([128, maxic_w], bf, tag="ind",
                                      name=f"ind2_{wi}_{h}")
                    nc.scalar.dma_start(ind_t[:, :icn],
                                        ind_d[:, ic0:ic0 + icn])
                    ps = pch.tile([128, 512], f32, tag="ps")
                    nc.tensor.matmul(ps[:, :wsz], zl_t[:], zr_t[:, :wsz],
                                     start=True, stop=False,
                                     skip_group_check=True)
                    for cj, (seg0, nn, coff) in enumerate(
                            struct['chunks'][wi][h]):
                        ch_i = ch_cum + cj
                        nc.tensor.matmul(
                            ps[:, seg0:seg0 + nn],
                            gts[op_base[h] + ch_i // GRP][
                                :, (ch_i % GRP) * 128:
                                (ch_i % GRP + 1) * 128],
                            ind_t[:, coff - ic0:coff - ic0 + nn],
                            start=False, stop=(cj == nch - 1),
                            skip_group_check=True)
                    ch_cum += nch
                    if h == 0:
                        nc.scalar.activation(s_t[:, w0:w0 + wsz], ps[:, :wsz],
                                             AF.Copy)
                    elif h == 1:
                        nc.vector.tensor_tensor(
                            s_t[:, w0:w0 + wsz], s_t[:, w0:w0 + wsz],
                            ps[:, :wsz], mybir.AluOpType.add)
                    else:
                        nc.vector.tensor_tensor(
                            s_t[:, w0:w0 + wsz], s_t[:, w0:w0 + wsz],
                            ps[:, :wsz], mybir.AluOpType.add)
                        # tail: xform, fet accumulate, h2 store
                        xp = px.tile([128, 512], f32, tag="xp",
                                     name=f"xp2_{wi}")
                        nc.tensor.matmul(xp[:, :wsz], w2_t[:],
                                         s_t[:, w0:w0 + wsz],
                                         start=True, stop=True)
                        h2w = h2p.tile([128, 512], f32, tag="h2w")
                        nc.scalar.activation(h2w[:, :wsz], xp[:, :wsz],
                                             AF.Relu, bias=bn_t[:, 1:2],
                                             scale=1.0)
                        fet_t = fetp.tile([128, 512], f32, tag="fetw")
                        nc.vector.tensor_tensor(
                            fet_t[:, :wsz], et_t[:, :wsz], h2w[:, :wsz],
                            mybir.AluOpType.add)
                        nc.scalar.dma_start(fet_d[:, w0:w0 + wsz],
                                            fet_t[:, :wsz])
                        h2b = h2bp.tile([128, 512], bf, tag="h2b")
                        nc.vector.tensor_copy(h2b[:, :wsz], h2w[:, :wsz])
                        if wi == NW - 1:
                            nc.vector.memset(h2b[:, wsz - 1:wsz], 0.0)
                        store_ht(h2b, hsh2, w0, wsz, 2)

            # ---------- batch tail ----------
            tlo = bp.tile([128, B], bf)
            nc.gpsimd.dma_gather(
                tlo[:].rearrange("p (c e) -> p c e", e=D), hsh2[:],
                bidx_t[:, 0:64], 1024, 1024, D, queue_num=0)
            xin_dram = dp.tile([128, B], bf, tag="xin")
            xout_dram = dp.tile([128, B], bf, tag="xout", addr_space="Shared")
            nc.sync.dma_start(xin_dram[:], tlo[:])
            # bf16 AllReduce is exact here: each slot has one real
            # contributor (its owner); the other 7 cores add the zeroed
            # pad row NSH-1.
            nc.gpsimd.collective_compute(
                "AllReduce", mybir.AluOpType.add,
                replica_groups=[list(range(NC))],
                ins=[xin_dram[:].opt()], outs=[xout_dram[:].opt()])
            xag = bp.tile([128, B], bf)
            nc.sync.dma_start(xag[:], xout_dram[:])
            xraw = bp.tile([128, B], f32)
            nc.vector.tensor_tensor(xraw[:], xag[:], ebh_t[:],
                                    mybir.AluOpType.add)
            xtb = bp.tile([128, B], f32)
            for j in range(8):
                tp = px.tile([128, 512], f32, tag="xp", name=f"bt_{j}")
                nc.tensor.transpose(tp[:, 0:128],
                                    xraw[:, j * 128:(j + 1) * 128], idf_t[:])
                nc.vector.tensor_scalar(
                    xtb[:, j * 128:(j + 1) * 128], tp[:, 0:128],
                    bn_t[:, 2:3], bn_t[:, 3:4],
                    mybir.AluOpType.mult, mybir.AluOpType.add)
            vmt = bp.tile([128, B], f32)
            for hb in range(2):
                sl = slice(hb * 512, hb * 512 + 512)
                wmp = px.tile([128, 512], f32, tag="xp", name=f"wm_{hb}")
                nc.tensor.matmul(wmp[:], w_t[:], rgt_t[:, sl],
                                 start=True, stop=True)
                nc.vector.tensor_tensor(vmt[:, sl], xtb[:, sl], wmp[:],
                                        mybir.AluOpType.mult)
            nc.vector.tensor_scalar(vmt[:], vmt[:], bn_t[:, 4:5], bn_t[:, 5:6],
                                    mybir.AluOpType.mult, mybir.AluOpType.add)

            # ---------- scoring ----------
            for wi, (w0, wsz) in enumerate(WINDOWS):
                fet_s = fsp.tile([128, 512], f32, tag="fets")
                nc.sync.dma_start(fet_s[:, :wsz], fet_d[:, w0:w0 + wsz])
                ob_w = obp.tile([128, 8 * 512], bf, tag="ob")
                for bt in range(8):
                    sc = psc.tile([128, 512], f32, tag="sc")
                    nc.tensor.matmul(sc[:, :wsz],
                                     vmt[:, bt * 128:(bt + 1) * 128],
                                     fet_s[:, :wsz],
                                     start=True, stop=True)
                    nc.scalar.activation(ob_w[:, bt * 512:bt * 512 + wsz],
                                         sc[:, :wsz], AF.Sigmoid)
                    if wsz < 512:
                        nc.vector.memset(
                            ob_w[:, bt * 512 + wsz:(bt + 1) * 512], 0.0)
                nc.sync.dma_start(out_d[:, wi * 4096:(wi + 1) * 4096], ob_w[:])
    nc.compile()
    return nc


def _host_prep(inputs):
    rows = np.asarray(inputs["adj_rows"]).astype(np.int64)
    cols = np.asarray(inputs["adj_cols"]).astype(np.int64)
    vals = np.asarray(inputs["adj_vals"], np.float32)
    E = np.asarray(inputs["E_emb"], np.float32)[np.asarray(inputs["init_ind"])]
    E_bf = E.astype(BF16)
    bh = np.asarray(inputs["batch_head"]).astype(np.int64)
    rel = np.asarray(inputs["batch_rel"]).astype(np.int64)
    R = np.asarray(inputs["R_emb"], np.float32)

    g0 = np.asarray(inputs["bn0_gamma"], np.float32) / np.sqrt(1.0 + BN_EPS)
    b0 = np.asarray(inputs["bn0_beta"], np.float32)
    g1 = np.asarray(inputs["bn1_gamma"], np.float32) / np.sqrt(1.0 + BN_EPS)
    b1v = np.asarray(inputs["bn1_beta"], np.float32)
    bn = np.ascontiguousarray(np.stack(
        [np.asarray(inputs["b1"], np.float32),
         np.asarray(inputs["b2"], np.float32),
         g0, b0, g1, b1v,
         np.zeros(D, np.float32), np.zeros(D, np.float32)], axis=1))

    bh_owner = bh // SH
    bh_local = bh - bh_owner * SH

    def slot_layout(a):          # [1024, D] -> [128, 8*D], slot i=(p,j)->j*128+p
        return np.ascontiguousarray(
            a.reshape(8, 128, D).transpose(1, 0, 2).reshape(128, 8 * D))

    ebh_l = slot_layout(E[bh])
    rgt = np.ascontiguousarray(R[rel].T.astype(np.float32))

    struct, cores = _make_plan(rows, cols, vals)
    ncht = struct['ncht']

    in_maps = []
    for k in range(NC):
        pl = cores[k]
        # L1 feature blob: [128 lanes, ncht, 128 dims], zeros on padding
        srcc = pl['g1src']
        g1 = np.zeros((ncht, 128, 128), BF16)
        m = srcc >= 0
        g1[m] = E_bf[srcc[m]]
        g1 = np.ascontiguousarray(
            g1.transpose(1, 0, 2).reshape(128, ncht * 128))
        et = np.zeros((D, NSH), np.float32)
        et[:, :SH] = E[k * SH:(k + 1) * SH].T
        in_maps.append({
            "g1": g1,
            "ind": pl['ind'],
            "idx": pl['idx'],
            "w1": np.asarray(inputs["W1"], np.float32).astype(BF16),
            "w2": np.asarray(inputs["W2"], np.float32).astype(BF16),
            "w": np.asarray(inputs["W"], np.float32),
            "bn": bn, "et": et, "ebh": ebh_l, "rgt": rgt,
            "bidx": _wrap_idx(np.where(bh_owner == k,
                                       _loc2slot(bh_local), NSH - 1)),
            "idb": np.eye(D, dtype=np.float32).astype(BF16),
            "idf": np.eye(D, dtype=np.float32),
        })
    return struct, in_maps


def _run(inputs, trace=False):
    struct, in_maps = _host_prep(inputs)
    nc = _build_nc(struct)
    res = run_bass_kernel_spmd(nc, in_maps, core_ids=list(range(NC)),
                               trace=trace)
    outs = []
    for k in range(NC):
        o = res.results[k]["out"]            # [128, NW*8*512] bf16
        o = o.reshape(128, NW, 8, 512).transpose(2, 0, 1, 3)  # [bt, p, wi, c]
        o = o.reshape(B, NW * 512)[:, :SH]
        outs.append(o)
    return np.concatenate(outs, axis=1).astype(np.float32), res


def kernel(**inputs):
    out, _ = _run(inputs, trace=False)
    return out
